# revision 29
# baseline (speedup 1.0000x reference)
"""Trainium2 Bass kernel for nn_NURQVAE_15745350107776 (vq_codebook).

Pure data-parallel over 8 NeuronCores: batch dim of x sharded 2048 rows/core,
MLP weights + codebooks replicated. Per core everything runs feature-major
(features on partitions, batch on the free dim) so every matmul contracts over
the partition axis; x / out are transposed at the edges with PE transposes.

RVQ per level: score[b,k] = 2*r.c - ||c||^2 via one PE matmul + DVE add,
argmax via DVE max/max_index, one-hot gather back through the PE.
Encoder + VQ run in true fp32 (argmin near-ties rule out lower precision);
decoder matmuls run as tf32 (float32r) whose rounding noise is far below this
problem's fp32 argmin-tie envelope.

Emission is split into front(chunk)=enc+VQ+kuma and back(chunk)=dec+store so
the scheduler fills each chunk's VQ valley with the other chunk's PE work.
Loss partial sums are returned per-core and reduced on the host (the only
"collective" this problem needs).
"""

import sys
import numpy as np

try:
    import concourse  # noqa: F401
except ImportError:  # grading env fallback
    sys.path.insert(0, "/opt/trn_rl_repo")

import concourse.bass as bass
import concourse.mybir as mybir
import concourse.tile as tile
from concourse import bacc
from concourse.alu_op_type import AluOpType as Alu
from concourse.bass import ts

P = 128
B_FULL = 16384
NCORES = 8
BC = B_FULL // NCORES  # 2048 rows per core
NCHUNK_COLS = 512      # batch columns per chunk (feature-major free dim)
ENC = [768, 2048, 1024, 512, 128]
DEC = [128, 512, 1024, 2048, 768]
EPS = 1e-6
BETA = 0.25

f32 = mybir.dt.float32
f32r = mybir.dt.float32r
bf16 = mybir.dt.bfloat16
i32 = mybir.dt.int32
u32 = mybir.dt.uint32
AF = mybir.ActivationFunctionType
AX = mybir.AxisListType

_ONE_SET_ONLY = True  # keep exp+ln in natural_log_exp_and_others


def _patch_act_tables():
    """Hide redundant exp/ln-bearing table sets from the set chooser so every
    activation lands in natural_log_exp_and_others (which also has relu /
    copy / identity / square). Index-preserving: set ids stay aligned with
    act_info.json, only the advertised contents shrink."""
    from concourse import hw_specs
    if getattr(hw_specs, "_nurq_patched", False):
        return
    orig = hw_specs.get_activation_tables

    def patched(arch):
        tables = dict(orig(arch))
        keep = tables.get("natural_log_exp_and_others")
        if keep:
            for name in ("exp_and_others", "exp_and_friends", "natural_log"):
                if name in tables:
                    tables[name] = set()
        return tables

    hw_specs.get_activation_tables = patched
    hw_specs._nurq_patched = True
    import concourse.bacc as bacc_mod
    if hasattr(bacc_mod, "get_activation_tables"):
        bacc_mod.get_activation_tables = patched


# --------------------------------------------------------------------------
# device program
# --------------------------------------------------------------------------

def build_program(bc=BC):
    if _ONE_SET_ONLY:
        _patch_act_tables()
    nc = bacc.Bacc("TRN2", target_bir_lowering=False, debug=False,
                   num_devices=NCORES)
    d = {}
    d["x"] = nc.dram_tensor("x", [bc, 768], f32, kind="ExternalInput")
    for i in range(4):
        m = ENC[i + 1] // P
        if i < 2:
            d[f"ewh{i}"] = nc.dram_tensor(f"ewh{i}", [m, P, ENC[i]], bf16,
                                          kind="ExternalInput")
            d[f"ewl{i}"] = nc.dram_tensor(f"ewl{i}", [m, P, ENC[i]], bf16,
                                          kind="ExternalInput")
        else:
            d[f"ew{i}"] = nc.dram_tensor(f"ew{i}", [m, P, ENC[i]], f32,
                                         kind="ExternalInput")
        d[f"eb{i}"] = nc.dram_tensor(f"eb{i}", [P, m], f32,
                                     kind="ExternalInput")
        m = DEC[i + 1] // P
        d[f"dw{i}"] = nc.dram_tensor(f"dw{i}", [m, P, DEC[i]], bf16,
                                     kind="ExternalInput")
        d[f"db{i}"] = nc.dram_tensor(f"db{i}", [P, m], f32,
                                     kind="ExternalInput")
    d["cbT2"] = nc.dram_tensor("cbT2", [4, P, 256], f32, kind="ExternalInput")
    d["cbN"] = nc.dram_tensor("cbN", [4, 256, P], f32, kind="ExternalInput")
    d["nc2b"] = nc.dram_tensor("nc2b", [4, P, 256], f32, kind="ExternalInput")
    d["abp"] = nc.dram_tensor("abp", [P, 4], f32, kind="ExternalInput")
    d["ident"] = nc.dram_tensor("ident", [P, P], f32, kind="ExternalInput")

    d["out"] = nc.dram_tensor("out", [bc, 768], f32, kind="ExternalOutput")
    d["indices"] = nc.dram_tensor("indices", [bc, 4], i32,
                                  kind="ExternalOutput")
    d["losses"] = nc.dram_tensor("losses", [P, 8], f32, kind="ExternalOutput")

    with tile.TileContext(nc) as tc:
        _emit(tc, nc, d, bc)
    nc.compile()
    return nc


def _emit(tc, nc, d, bc):
    from contextlib import ExitStack
    N = NCHUNK_COLS
    nchunks = bc // N
    NH = N // 512

    ctx = ExitStack()
    with ctx:
        const = ctx.enter_context(tc.tile_pool(name="const", bufs=1))
        poolA = ctx.enter_context(tc.tile_pool(name="A", bufs=12))
        poolB = ctx.enter_context(tc.tile_pool(name="B", bufs=17))
        poolZ = ctx.enter_context(tc.tile_pool(name="Z", bufs=7))
        poolHE = ctx.enter_context(tc.tile_pool(name="HE", bufs=42))
        poolHF = ctx.enter_context(tc.tile_pool(name="HF", bufs=8))
        xst = ctx.enter_context(tc.tile_pool(name="xst", bufs=2))
        ost = ctx.enter_context(tc.tile_pool(name="ost", bufs=2))
        vq = ctx.enter_context(tc.tile_pool(name="vq", bufs=2))
        wpools = {
            fin: ctx.enter_context(tc.tile_pool(name=f"w{fin}", bufs=2))
            for fin in (768, 2048, 1024, 512, 128)
        }
        wbpools = {
            768: ctx.enter_context(tc.tile_pool(name="wb768", bufs=4)),
            2048: ctx.enter_context(tc.tile_pool(name="wb2048", bufs=4)),
        }
        PSUM = bass.MemorySpace.PSUM
        pmm = ctx.enter_context(tc.tile_pool(name="pmm", bufs=4, space=PSUM))
        ppt = ctx.enter_context(tc.tile_pool(name="ppt", bufs=1, space=PSUM))
        pvt = ctx.enter_context(tc.tile_pool(name="pvt", bufs=1, space=PSUM))
        pvs = ctx.enter_context(tc.tile_pool(name="pvs", bufs=1, space=PSUM))
        pvq = ctx.enter_context(tc.tile_pool(name="pvq", bufs=1, space=PSUM))

        # ---- persistent constants ----
        ident = const.tile([P, P], f32, tag="ident", name="ident")
        nc.sync.dma_start(ident[:], d["ident"][:])
        abp = const.tile([P, 4], f32, tag="abp", name="abp")
        nc.sync.dma_start(abp[:], d["abp"][:])
        a_ap, b_ap = abp[:, 0:1], abp[:, 1:2]
        ia_ap, ib_ap = abp[:, 2:3], abp[:, 3:4]
        cbt2, nc2b, cbn0, cbn1 = [], [], [], []
        for l in range(4):
            t = const.tile([P, 256], f32, tag=f"cbt2_{l}", name=f"cbt2_{l}")
            nc.sync.dma_start(t[:], d["cbT2"][l])
            cbt2.append(t)
            t = const.tile([P, 256], f32, tag=f"nc2b_{l}", name=f"nc2b_{l}")
            nc.sync.dma_start(t[:], d["nc2b"][l])
            nc2b.append(t)
            t = const.tile([P, P], f32, tag=f"cbn0_{l}", name=f"cbn0_{l}")
            nc.sync.dma_start(t[:], d["cbN"][l, 0:128, :])
            cbn0.append(t)
            t = const.tile([P, P], f32, tag=f"cbn1_{l}", name=f"cbn1_{l}")
            nc.sync.dma_start(t[:], d["cbN"][l, 128:256, :])
            cbn1.append(t)
        ebt, dbt = [], []
        for i in range(4):
            t = const.tile([P, ENC[i + 1] // P], f32, tag=f"eb{i}",
                           name=f"eb{i}")
            nc.sync.dma_start(t[:], d[f"eb{i}"][:])
            ebt.append(t)
            t = const.tile([P, DEC[i + 1] // P], f32, tag=f"db{i}",
                           name=f"db{i}")
            nc.sync.dma_start(t[:], d[f"db{i}"][:])
            dbt.append(t)
        sse = const.tile([P, 8], f32, tag="sse", name="sse")
        nc.vector.memset(sse[:], 0.0)

        def tile_A(dt=f32):
            return poolA.tile([P, N], dt, tag="A", name="A")

        def tile_B(dt=f32):
            return poolB.tile([P, N], dt, tag="B", name="B")

        def tile_Z(dt=f32):
            return poolZ.tile([P, N], dt, tag="Z", name="Z")

        def to_pair(src_tile):
            """f32 tile -> (hi, lo) bf16 pair; hi = bf16(x), lo = bf16(x-hi)."""
            hi = poolHE.tile([P, N], bf16, tag="HE", name="HE")
            nc.vector.tensor_copy(hi[:], src_tile[:])
            lo = poolHE.tile([P, N], bf16, tag="HE", name="HE")
            nc.vector.tensor_sub(lo[:], src_tile[:], hi[:])
            return (hi, lo)

        def mlp_layer(h_in, wname, bias, fin, fout, relu, out_alloc,
                      tf32=False):
            K, M = fin // P, fout // P
            outs = []
            wdt = bf16 if tf32 else f32
            for m in range(M):
                wt = wpools[fin].tile([P, fin], wdt, tag=f"w{fin}",
                                      name=f"w{fin}")
                nc.sync.dma_start(wt[:], d[wname][m])
                ps = pmm.tile([P, N], f32, tag="mm", name="mm")
                for kt in range(K):
                    lhsT = wt[:, ts(kt, P)]
                    for nh in range(NH):
                        rhs = h_in[kt][:, ts(nh, 512)]
                        nc.tensor.matmul(
                            ps[:, ts(nh, 512)], lhsT, rhs,
                            start=(kt == 0), stop=(kt == K - 1))
                ot = out_alloc()
                nc.scalar.activation(ot[:], ps[:],
                                     AF.Relu if relu else AF.Identity,
                                     bias=bias[:, m:m + 1])
                outs.append(ot)
            return outs

        def mlp_split_layer(h_pairs, i, bias, fin, fout, relu,
                            out_pairs, out_alloc=None):
            """3-term bf16 split: ps += Wh.hh + Wh.hl + Wl.hh.
            h_pairs: list of (hi, lo) bf16 tiles. Returns bf16 pairs
            (out_pairs=True, via transient HF tiles) or persistent f32
            tiles from out_alloc."""
            K, M = fin // P, fout // P
            outs = []
            for m in range(M):
                wh = wbpools[fin].tile([P, fin], bf16, tag=f"wb{fin}",
                                       name=f"wb{fin}")
                nc.sync.dma_start(wh[:], d[f"ewh{i}"][m])
                wl = wbpools[fin].tile([P, fin], bf16, tag=f"wb{fin}",
                                       name=f"wb{fin}")
                nc.sync.dma_start(wl[:], d[f"ewl{i}"][m])
                ps = pmm.tile([P, N], f32, tag="mm", name="mm")
                for kt in range(K):
                    hh, hl = h_pairs[kt]
                    terms = ((wh, hh), (wh, hl), (wl, hh))
                    for tix, (wmat, hmat) in enumerate(terms):
                        nc.tensor.matmul(
                            ps[:], wmat[:, ts(kt, P)], hmat[:],
                            start=(kt == 0 and tix == 0),
                            stop=(kt == K - 1 and tix == 2))
                if out_pairs:
                    hf = poolHF.tile([P, N], f32, tag="HF", name="HF")
                else:
                    hf = out_alloc()
                nc.scalar.activation(hf[:], ps[:],
                                     AF.Relu if relu else AF.Identity,
                                     bias=bias[:, m:m + 1])
                outs.append(to_pair(hf) if out_pairs else hf)
            return outs

        def kuma_inv(y_tile, out_tile):
            # out = logit(clip((1 - clip(1-y)^(1/b))^(1/a)))
            t = tile_Z()
            nc.vector.tensor_scalar(t[:], y_tile[:], EPS, 1.0 - EPS,
                                    Alu.max, Alu.min)
            nc.scalar.activation(t[:], t[:], AF.Copy, bias=1.0, scale=-1.0)
            nc.scalar.activation(t[:], t[:], AF.Ln)
            nc.scalar.activation(t[:], t[:], AF.Exp, scale=ib_ap)
            nc.scalar.activation(t[:], t[:], AF.Copy, bias=1.0, scale=-1.0)
            nc.vector.tensor_scalar(t[:], t[:], EPS, 1.0 - EPS,
                                    Alu.max, Alu.min)
            nc.scalar.activation(t[:], t[:], AF.Ln)
            nc.scalar.activation(t[:], t[:], AF.Exp, scale=ia_ap)
            nc.vector.tensor_scalar(t[:], t[:], EPS, 1.0 - EPS,
                                    Alu.max, Alu.min)
            u = tile_Z()
            nc.scalar.activation(u[:], t[:], AF.Copy, bias=1.0, scale=-1.0)
            nc.scalar.activation(u[:], u[:], AF.Ln)
            nc.scalar.activation(t[:], t[:], AF.Ln)
            nc.vector.tensor_sub(out_tile[:], t[:], u[:])

        def emit_front(c):
            """x load + transpose, encoder, kuma fwd, VQ, kuma inv.
            Returns the zq tile (decoder input)."""
            row0 = c * N

            h0f = [poolHF.tile([P, N], f32, tag="HF", name="HF")
                   for _ in range(6)]
            for bt in range(N // P):
                xa = xst.tile([P, 768], f32, tag="xst", name="xst")
                nc.sync.dma_start(
                    xa[:], d["x"][row0 + bt * P: row0 + (bt + 1) * P, :])
                for ft in range(6):
                    pst = ppt.tile([P, P], f32, tag="pt", name="pt")
                    nc.tensor.transpose(pst[:], xa[:, ts(ft, P)], ident[:])
                    nc.vector.tensor_copy(h0f[ft][:, ts(bt, P)], pst[:])
            h0 = [to_pair(t) for t in h0f]

            h1 = mlp_split_layer(h0, 0, ebt[0], 768, 2048, True, True)
            h2 = mlp_split_layer(h1, 1, ebt[1], 2048, 1024, True, False,
                                 tile_A)
            h3 = mlp_layer(h2, "ew2", ebt[2], 1024, 512, True, tile_B)
            z = mlp_layer(h3, "ew3", ebt[3], 512, 128, False, tile_Z)[0]

            # kuma forward: z' = 1 - (1 - sigmoid(z)^a)^b, clipped
            t = tile_Z()
            nc.scalar.activation(t[:], z[:], AF.Exp, scale=-1.0)
            nc.scalar.activation(t[:], t[:], AF.Identity, bias=1.0)
            nc.vector.reciprocal(t[:], t[:])
            nc.vector.tensor_scalar(t[:], t[:], EPS, 1.0 - EPS,
                                    Alu.max, Alu.min)
            nc.scalar.activation(t[:], t[:], AF.Ln)
            nc.scalar.activation(t[:], t[:], AF.Exp, scale=a_ap)
            nc.scalar.activation(t[:], t[:], AF.Copy, bias=1.0, scale=-1.0)
            nc.vector.tensor_scalar(t[:], t[:], EPS, None, Alu.max)
            nc.scalar.activation(t[:], t[:], AF.Ln)
            nc.scalar.activation(t[:], t[:], AF.Exp, scale=b_ap)
            zp = tile_Z()
            nc.scalar.activation(zp[:], t[:], AF.Copy, bias=1.0, scale=-1.0)
            nc.vector.tensor_scalar(zp[:], zp[:], EPS, 1.0 - EPS,
                                    Alu.max, Alu.min)

            # residual VQ, 4 levels
            r = tile_Z()
            nc.vector.tensor_copy(r[:], zp[:])
            xq = tile_Z()
            for sc in range(N // P):
                scs = ts(sc, P)
                idx_t = vq.tile([P, 4], u32, tag="idxt", name="idxt")
                for l in range(4):
                    ps_s = pvs.tile([P, 256], f32, tag="ps_s", name="ps_s")
                    nc.tensor.matmul(ps_s[:], r[:, scs], cbt2[l][:],
                                     start=True, stop=True)
                    s_sb = vq.tile([P, 256], f32, tag="s_sb", name="s_sb")
                    nc.vector.tensor_tensor(s_sb[:], ps_s[:], nc2b[l][:],
                                            Alu.add)
                    mx8 = vq.tile([P, 8], f32, tag="mx8", name="mx8")
                    nc.vector.max(mx8[:], s_sb[:])
                    ix8 = vq.tile([P, 8], u32, tag="ix8", name="ix8")
                    nc.vector.max_index(ix8[:], mx8[:], s_sb[:])
                    nc.vector.tensor_copy(idx_t[:, l:l + 1], ix8[:, 0:1])
                    eq = vq.tile([P, 256], f32, tag="eq", name="eq")
                    nc.vector.tensor_scalar(eq[:], s_sb[:], mx8[:, 0:1],
                                            None, Alu.is_equal)
                    eqT = []
                    for j in range(2):
                        pst = pvt.tile([P, P], f32, tag="vt", name="vt")
                        nc.tensor.transpose(pst[:], eq[:, ts(j, P)], ident[:])
                        et = vq.tile([P, P], f32, tag=f"eqT{j}",
                                     name=f"eqT{j}")
                        nc.vector.tensor_copy(et[:], pst[:])
                        eqT.append(et)
                    ps_q = pvq.tile([P, P], f32, tag="ps_q", name="ps_q")
                    nc.tensor.matmul(ps_q[:], cbn0[l][:], eqT[0][:],
                                     start=True, stop=False)
                    nc.tensor.matmul(ps_q[:], cbn1[l][:], eqT[1][:],
                                     start=False, stop=True)
                    q_sb = vq.tile([P, P], f32, tag="q_sb", name="q_sb")
                    nc.vector.tensor_copy(q_sb[:], ps_q[:])
                    if l == 0:
                        nc.vector.tensor_copy(xq[:, scs], q_sb[:])
                    else:
                        nc.vector.tensor_add(xq[:, scs], xq[:, scs], q_sb[:])
                    nc.vector.tensor_sub(r[:, scs], r[:, scs], q_sb[:])
                    r2 = vq.tile([P, P], f32, tag="r2", name="r2")
                    nc.scalar.square(r2[:], r[:, scs])
                    red = vq.tile([P, 1], f32, tag="red", name="red")
                    nc.vector.tensor_reduce(red[:], r2[:], axis=AX.X,
                                            op=Alu.add)
                    nc.vector.tensor_add(sse[:, l:l + 1], sse[:, l:l + 1],
                                         red[:])
                nc.sync.dma_start(
                    d["indices"][row0 + sc * P: row0 + (sc + 1) * P, :],
                    idx_t[:].bitcast(i32))

            # kuma inverse on xq (z_q) and z' (z_recon); nvq loss
            zq = tile_Z(bf16)
            kuma_inv(xq, zq)
            zrec = tile_Z()
            kuma_inv(zp, zrec)
            dn = tile_Z()
            nc.vector.tensor_sub(dn[:], zrec[:], z[:])
            nc.scalar.square(dn[:], dn[:])
            redn = vq.tile([P, 1], f32, tag="red", name="red")
            nc.vector.tensor_reduce(redn[:], dn[:], axis=AX.X, op=Alu.add)
            nc.vector.tensor_add(sse[:, 4:5], sse[:, 4:5], redn[:])
            return zq

        def emit_back(c, zq):
            """decoder (tf32) + transpose-and-store."""
            row0 = c * N
            tile_Br = lambda: tile_B(bf16)
            tile_Ar = lambda: tile_A(bf16)
            g0 = mlp_layer([zq], "dw0", dbt[0], 128, 512, True, tile_Br,
                           tf32=True)
            g1 = mlp_layer(g0, "dw1", dbt[1], 512, 1024, True, tile_Ar,
                           tf32=True)
            g2 = mlp_layer(g1, "dw2", dbt[2], 1024, 2048, True, tile_Br,
                           tf32=True)
            om = mlp_layer(g2, "dw3", dbt[3], 2048, 768, False, tile_A,
                           tf32=True)

            for bt in range(N // P):
                ot = ost.tile([P, 768], f32, tag="ost", name="ost")
                for ftm in range(6):
                    pst = ppt.tile([P, P], f32, tag="pt", name="pt")
                    nc.tensor.transpose(pst[:], om[ftm][:, ts(bt, P)],
                                        ident[:])
                    nc.vector.tensor_copy(ot[:, ts(ftm, P)], pst[:])
                nc.sync.dma_start(
                    d["out"][row0 + bt * P: row0 + (bt + 1) * P, :], ot[:])

        zqs = [emit_front(c) for c in range(nchunks)]
        for c in range(nchunks):
            emit_back(c, zqs[c])

        nc.sync.dma_start(d["losses"][:], sse[:])


# --------------------------------------------------------------------------
# host side
# --------------------------------------------------------------------------

def _stripes(w):
    """[fin, fout] -> [fout//128, 128, fin] so device weight DMAs are
    contiguous and w_stripe[m][:, kt*128:(kt+1)*128] is the lhsT k-tile."""
    fin, fout = w.shape
    return np.ascontiguousarray(
        w.reshape(fin // P, P, fout // P, P).transpose(2, 1, 0, 3)
        .reshape(fout // P, P, fin))


def _bias_cols(b):
    return np.ascontiguousarray(b.reshape(-1, P).T)


def _tf32_round(w):
    """Round-to-nearest-even fp32 -> tf32 (10-bit mantissa) bit pattern."""
    u = w.view(np.uint32)
    r = (u + np.uint32(0xFFF) + ((u >> np.uint32(13)) & np.uint32(1)))
    return (r & np.uint32(0xFFFFE000)).view(np.float32)


def prep_inputs(inputs):
    inp = {k: np.ascontiguousarray(np.asarray(v)) for k, v in inputs.items()}
    cb = inp["codebooks"].astype(np.float32)
    shared = {}
    import ml_dtypes
    for i in range(4):
        w = inp[f"enc_w{i}"].astype(np.float32)
        if i < 2:  # split layers: bf16 hi/lo weight pairs
            wh = w.astype(ml_dtypes.bfloat16)
            wl = (w - wh.astype(np.float32)).astype(ml_dtypes.bfloat16)
            shared[f"ewh{i}"] = _stripes(wh)
            shared[f"ewl{i}"] = _stripes(wl)
        else:
            shared[f"ew{i}"] = _stripes(w)
        shared[f"eb{i}"] = _bias_cols(inp[f"enc_b{i}"].astype(np.float32))
        shared[f"dw{i}"] = _stripes(
            inp[f"dec_w{i}"].astype(np.float32).astype(ml_dtypes.bfloat16))
        shared[f"db{i}"] = _bias_cols(inp[f"dec_b{i}"].astype(np.float32))
    shared["cbT2"] = np.ascontiguousarray(
        (2.0 * cb).transpose(0, 2, 1)).astype(np.float32)
    shared["cbN"] = cb
    c2 = (cb * cb).sum(-1)  # [4, 256] fp32
    shared["nc2b"] = np.ascontiguousarray(
        np.broadcast_to((-c2)[:, None, :], (4, P, 256))).astype(np.float32)
    a_raw = inp["a_raw"].astype(np.float32)
    b_raw = inp["b_raw"].astype(np.float32)
    a = np.log1p(np.exp(a_raw)).astype(np.float32) + np.float32(EPS)
    b = np.log1p(np.exp(b_raw)).astype(np.float32) + np.float32(EPS)
    shared["abp"] = np.stack(
        [a, b, (np.float32(1.0) / a), (np.float32(1.0) / b)],
        axis=1).astype(np.float32)
    shared["ident"] = np.eye(P, dtype=np.float32)
    return inp["x"].astype(np.float32), shared


def combine_outputs(results, bc, btot):
    out = np.concatenate([r["out"] for r in results], axis=0)
    indices = np.concatenate([r["indices"] for r in results], axis=0)
    sse = np.zeros((P, 8), np.float64)
    for r in results:
        sse += r["losses"].astype(np.float64)
    nel = float(btot * 128)
    rq = np.mean([(1.0 + BETA) * sse[:, l].sum() / nel for l in range(4)])
    nvq = sse[:, 4].sum() / nel
    loss = np.float32(rq + nvq)
    return out, loss, indices.astype(np.int32)


def run(inputs, trace=False):
    from concourse.bass_utils import run_bass_kernel_spmd
    x, shared = prep_inputs(inputs)
    nc = build_program(BC)
    in_maps = []
    for c in range(NCORES):
        m = dict(shared)
        m["x"] = np.ascontiguousarray(x[c * BC:(c + 1) * BC])
        in_maps.append(m)
    res = run_bass_kernel_spmd(nc, in_maps, list(range(NCORES)), trace=trace)
    out, loss, indices = combine_outputs(res.results, BC, B_FULL)
    return (out, loss, indices), res, nc


def kernel(**inputs):
    (out, loss, indices), _, _ = run(inputs, trace=False)
    return out, loss, indices


# revision 30
# speedup vs baseline: 1.0030x; 1.0030x over previous
"""Trainium2 Bass kernel for nn_NURQVAE_15745350107776 (vq_codebook).

Pure data-parallel over 8 NeuronCores: batch dim of x sharded 2048 rows/core,
MLP weights + codebooks replicated. Per core everything runs feature-major
(features on partitions, batch on the free dim) so every matmul contracts over
the partition axis; x / out are transposed at the edges with PE transposes.

RVQ per level: score[b,k] = 2*r.c - ||c||^2 via one PE matmul + DVE add,
argmax via DVE max/max_index, one-hot gather back through the PE.
Encoder + VQ run in true fp32 (argmin near-ties rule out lower precision);
decoder matmuls run as tf32 (float32r) whose rounding noise is far below this
problem's fp32 argmin-tie envelope.

Emission is split into front(chunk)=enc+VQ+kuma and back(chunk)=dec+store so
the scheduler fills each chunk's VQ valley with the other chunk's PE work.
Loss partial sums are returned per-core and reduced on the host (the only
"collective" this problem needs).
"""

import sys
import numpy as np

try:
    import concourse  # noqa: F401
except ImportError:  # grading env fallback
    sys.path.insert(0, "/opt/trn_rl_repo")

import concourse.bass as bass
import concourse.mybir as mybir
import concourse.tile as tile
from concourse import bacc
from concourse.alu_op_type import AluOpType as Alu
from concourse.bass import ts

P = 128
B_FULL = 16384
NCORES = 8
BC = B_FULL // NCORES  # 2048 rows per core
NCHUNK_COLS = 512      # batch columns per chunk (feature-major free dim)
ENC = [768, 2048, 1024, 512, 128]
DEC = [128, 512, 1024, 2048, 768]
EPS = 1e-6
BETA = 0.25

f32 = mybir.dt.float32
f32r = mybir.dt.float32r
bf16 = mybir.dt.bfloat16
i32 = mybir.dt.int32
u32 = mybir.dt.uint32
AF = mybir.ActivationFunctionType
AX = mybir.AxisListType

_ONE_SET_ONLY = True  # keep exp+ln in natural_log_exp_and_others


def _patch_act_tables():
    """Hide redundant exp/ln-bearing table sets from the set chooser so every
    activation lands in natural_log_exp_and_others (which also has relu /
    copy / identity / square). Index-preserving: set ids stay aligned with
    act_info.json, only the advertised contents shrink."""
    from concourse import hw_specs
    if getattr(hw_specs, "_nurq_patched", False):
        return
    orig = hw_specs.get_activation_tables

    def patched(arch):
        tables = dict(orig(arch))
        keep = tables.get("natural_log_exp_and_others")
        if keep:
            for name in ("exp_and_others", "exp_and_friends", "natural_log"):
                if name in tables:
                    tables[name] = set()
        return tables

    hw_specs.get_activation_tables = patched
    hw_specs._nurq_patched = True
    import concourse.bacc as bacc_mod
    if hasattr(bacc_mod, "get_activation_tables"):
        bacc_mod.get_activation_tables = patched


# --------------------------------------------------------------------------
# device program
# --------------------------------------------------------------------------

def build_program(bc=BC):
    if _ONE_SET_ONLY:
        _patch_act_tables()
    nc = bacc.Bacc("TRN2", target_bir_lowering=False, debug=False,
                   num_devices=NCORES)
    d = {}
    d["x"] = nc.dram_tensor("x", [bc, 768], f32, kind="ExternalInput")
    for i in range(4):
        m = ENC[i + 1] // P
        if i < 2:
            d[f"ewh{i}"] = nc.dram_tensor(f"ewh{i}", [m, P, ENC[i]], bf16,
                                          kind="ExternalInput")
            d[f"ewl{i}"] = nc.dram_tensor(f"ewl{i}", [m, P, ENC[i]], bf16,
                                          kind="ExternalInput")
        else:
            d[f"ew{i}"] = nc.dram_tensor(f"ew{i}", [m, P, ENC[i]], f32,
                                         kind="ExternalInput")
        d[f"eb{i}"] = nc.dram_tensor(f"eb{i}", [P, m], f32,
                                     kind="ExternalInput")
        m = DEC[i + 1] // P
        d[f"dw{i}"] = nc.dram_tensor(f"dw{i}", [m, P, DEC[i]], bf16,
                                     kind="ExternalInput")
        d[f"db{i}"] = nc.dram_tensor(f"db{i}", [P, m], f32,
                                     kind="ExternalInput")
    d["cbT2"] = nc.dram_tensor("cbT2", [4, P, 256], f32, kind="ExternalInput")
    d["cbN"] = nc.dram_tensor("cbN", [4, 256, P], f32, kind="ExternalInput")
    d["nc2b"] = nc.dram_tensor("nc2b", [4, P, 256], f32, kind="ExternalInput")
    d["abp"] = nc.dram_tensor("abp", [P, 4], f32, kind="ExternalInput")
    d["ident"] = nc.dram_tensor("ident", [P, P], f32, kind="ExternalInput")

    d["out"] = nc.dram_tensor("out", [bc, 768], f32, kind="ExternalOutput")
    d["indices"] = nc.dram_tensor("indices", [bc, 4], i32,
                                  kind="ExternalOutput")
    d["losses"] = nc.dram_tensor("losses", [P, 8], f32, kind="ExternalOutput")

    with tile.TileContext(nc) as tc:
        _emit(tc, nc, d, bc)
    nc.compile()
    return nc


def _emit(tc, nc, d, bc):
    from contextlib import ExitStack
    N = NCHUNK_COLS
    nchunks = bc // N
    NH = N // 512

    ctx = ExitStack()
    with ctx:
        const = ctx.enter_context(tc.tile_pool(name="const", bufs=1))
        poolA = ctx.enter_context(tc.tile_pool(name="A", bufs=12))
        poolB = ctx.enter_context(tc.tile_pool(name="B", bufs=17))
        poolZ = ctx.enter_context(tc.tile_pool(name="Z", bufs=6))
        poolHE = ctx.enter_context(tc.tile_pool(name="HE", bufs=41))
        poolHF = ctx.enter_context(tc.tile_pool(name="HF", bufs=7))
        xst = ctx.enter_context(tc.tile_pool(name="xst", bufs=2))
        ost = ctx.enter_context(tc.tile_pool(name="ost", bufs=2))
        vq = ctx.enter_context(tc.tile_pool(name="vq", bufs=2))
        wpools = {
            fin: ctx.enter_context(tc.tile_pool(name=f"w{fin}", bufs=b))
            for fin, b in ((768, 2), (2048, 3), (1024, 2), (512, 2), (128, 2))
        }
        wbpools = {
            768: ctx.enter_context(tc.tile_pool(name="wb768", bufs=4)),
            2048: ctx.enter_context(tc.tile_pool(name="wb2048", bufs=4)),
        }
        PSUM = bass.MemorySpace.PSUM
        pmm = ctx.enter_context(tc.tile_pool(name="pmm", bufs=4, space=PSUM))
        ppt = ctx.enter_context(tc.tile_pool(name="ppt", bufs=1, space=PSUM))
        pvt = ctx.enter_context(tc.tile_pool(name="pvt", bufs=1, space=PSUM))
        pvs = ctx.enter_context(tc.tile_pool(name="pvs", bufs=1, space=PSUM))
        pvq = ctx.enter_context(tc.tile_pool(name="pvq", bufs=1, space=PSUM))

        # ---- persistent constants ----
        ident = const.tile([P, P], f32, tag="ident", name="ident")
        nc.sync.dma_start(ident[:], d["ident"][:])
        abp = const.tile([P, 4], f32, tag="abp", name="abp")
        nc.sync.dma_start(abp[:], d["abp"][:])
        a_ap, b_ap = abp[:, 0:1], abp[:, 1:2]
        ia_ap, ib_ap = abp[:, 2:3], abp[:, 3:4]
        cbt2, nc2b, cbn0, cbn1 = [], [], [], []
        for l in range(4):
            t = const.tile([P, 256], f32, tag=f"cbt2_{l}", name=f"cbt2_{l}")
            nc.sync.dma_start(t[:], d["cbT2"][l])
            cbt2.append(t)
            t = const.tile([P, 256], f32, tag=f"nc2b_{l}", name=f"nc2b_{l}")
            nc.sync.dma_start(t[:], d["nc2b"][l])
            nc2b.append(t)
            t = const.tile([P, P], f32, tag=f"cbn0_{l}", name=f"cbn0_{l}")
            nc.sync.dma_start(t[:], d["cbN"][l, 0:128, :])
            cbn0.append(t)
            t = const.tile([P, P], f32, tag=f"cbn1_{l}", name=f"cbn1_{l}")
            nc.sync.dma_start(t[:], d["cbN"][l, 128:256, :])
            cbn1.append(t)
        ebt, dbt = [], []
        for i in range(4):
            t = const.tile([P, ENC[i + 1] // P], f32, tag=f"eb{i}",
                           name=f"eb{i}")
            nc.sync.dma_start(t[:], d[f"eb{i}"][:])
            ebt.append(t)
            t = const.tile([P, DEC[i + 1] // P], f32, tag=f"db{i}",
                           name=f"db{i}")
            nc.sync.dma_start(t[:], d[f"db{i}"][:])
            dbt.append(t)
        sse = const.tile([P, 8], f32, tag="sse", name="sse")
        nc.vector.memset(sse[:], 0.0)

        def tile_A(dt=f32):
            return poolA.tile([P, N], dt, tag="A", name="A")

        def tile_B(dt=f32):
            return poolB.tile([P, N], dt, tag="B", name="B")

        def tile_Z(dt=f32):
            return poolZ.tile([P, N], dt, tag="Z", name="Z")

        def to_pair(src_tile):
            """f32 tile -> (hi, lo) bf16 pair; hi = bf16(x), lo = bf16(x-hi)."""
            hi = poolHE.tile([P, N], bf16, tag="HE", name="HE")
            nc.vector.tensor_copy(hi[:], src_tile[:])
            lo = poolHE.tile([P, N], bf16, tag="HE", name="HE")
            nc.vector.tensor_sub(lo[:], src_tile[:], hi[:])
            return (hi, lo)

        def mlp_layer(h_in, wname, bias, fin, fout, relu, out_alloc,
                      tf32=False):
            K, M = fin // P, fout // P
            outs = []
            wdt = bf16 if tf32 else f32
            for m in range(M):
                wt = wpools[fin].tile([P, fin], wdt, tag=f"w{fin}",
                                      name=f"w{fin}")
                nc.sync.dma_start(wt[:], d[wname][m])
                ps = pmm.tile([P, N], f32, tag="mm", name="mm")
                for kt in range(K):
                    lhsT = wt[:, ts(kt, P)]
                    for nh in range(NH):
                        rhs = h_in[kt][:, ts(nh, 512)]
                        nc.tensor.matmul(
                            ps[:, ts(nh, 512)], lhsT, rhs,
                            start=(kt == 0), stop=(kt == K - 1))
                ot = out_alloc()
                nc.scalar.activation(ot[:], ps[:],
                                     AF.Relu if relu else AF.Identity,
                                     bias=bias[:, m:m + 1])
                outs.append(ot)
            return outs

        def mlp_split_layer(h_pairs, i, bias, fin, fout, relu,
                            out_pairs, out_alloc=None):
            """3-term bf16 split: ps += Wh.hh + Wh.hl + Wl.hh.
            h_pairs: list of (hi, lo) bf16 tiles. Returns bf16 pairs
            (out_pairs=True, via transient HF tiles) or persistent f32
            tiles from out_alloc."""
            K, M = fin // P, fout // P
            outs = []
            for m in range(M):
                wh = wbpools[fin].tile([P, fin], bf16, tag=f"wb{fin}",
                                       name=f"wb{fin}")
                nc.sync.dma_start(wh[:], d[f"ewh{i}"][m])
                wl = wbpools[fin].tile([P, fin], bf16, tag=f"wb{fin}",
                                       name=f"wb{fin}")
                nc.sync.dma_start(wl[:], d[f"ewl{i}"][m])
                ps = pmm.tile([P, N], f32, tag="mm", name="mm")
                for kt in range(K):
                    hh, hl = h_pairs[kt]
                    terms = ((wh, hh), (wh, hl), (wl, hh))
                    for tix, (wmat, hmat) in enumerate(terms):
                        nc.tensor.matmul(
                            ps[:], wmat[:, ts(kt, P)], hmat[:],
                            start=(kt == 0 and tix == 0),
                            stop=(kt == K - 1 and tix == 2))
                if out_pairs:
                    hf = poolHF.tile([P, N], f32, tag="HF", name="HF")
                else:
                    hf = out_alloc()
                nc.scalar.activation(hf[:], ps[:],
                                     AF.Relu if relu else AF.Identity,
                                     bias=bias[:, m:m + 1])
                outs.append(to_pair(hf) if out_pairs else hf)
            return outs

        def kuma_inv(y_tile, out_tile):
            # out = logit(clip((1 - clip(1-y)^(1/b))^(1/a)))
            t = tile_Z()
            nc.vector.tensor_scalar(t[:], y_tile[:], EPS, 1.0 - EPS,
                                    Alu.max, Alu.min)
            nc.scalar.activation(t[:], t[:], AF.Copy, bias=1.0, scale=-1.0)
            nc.scalar.activation(t[:], t[:], AF.Ln)
            nc.scalar.activation(t[:], t[:], AF.Exp, scale=ib_ap)
            nc.scalar.activation(t[:], t[:], AF.Copy, bias=1.0, scale=-1.0)
            nc.vector.tensor_scalar(t[:], t[:], EPS, 1.0 - EPS,
                                    Alu.max, Alu.min)
            nc.scalar.activation(t[:], t[:], AF.Ln)
            nc.scalar.activation(t[:], t[:], AF.Exp, scale=ia_ap)
            nc.vector.tensor_scalar(t[:], t[:], EPS, 1.0 - EPS,
                                    Alu.max, Alu.min)
            u = tile_Z()
            nc.scalar.activation(u[:], t[:], AF.Copy, bias=1.0, scale=-1.0)
            nc.scalar.activation(u[:], u[:], AF.Ln)
            nc.scalar.activation(t[:], t[:], AF.Ln)
            nc.vector.tensor_sub(out_tile[:], t[:], u[:])

        def emit_front(c):
            """x load + transpose, encoder, kuma fwd, VQ, kuma inv.
            Returns the zq tile (decoder input)."""
            row0 = c * N

            h0f = [poolHF.tile([P, N], f32, tag="HF", name="HF")
                   for _ in range(6)]
            for bt in range(N // P):
                xa = xst.tile([P, 768], f32, tag="xst", name="xst")
                nc.sync.dma_start(
                    xa[:], d["x"][row0 + bt * P: row0 + (bt + 1) * P, :])
                for ft in range(6):
                    pst = ppt.tile([P, P], f32, tag="pt", name="pt")
                    nc.tensor.transpose(pst[:], xa[:, ts(ft, P)], ident[:])
                    nc.vector.tensor_copy(h0f[ft][:, ts(bt, P)], pst[:])
            h0 = [to_pair(t) for t in h0f]

            h1 = mlp_split_layer(h0, 0, ebt[0], 768, 2048, True, True)
            h2 = mlp_split_layer(h1, 1, ebt[1], 2048, 1024, True, False,
                                 tile_A)
            h3 = mlp_layer(h2, "ew2", ebt[2], 1024, 512, True, tile_B)
            z = mlp_layer(h3, "ew3", ebt[3], 512, 128, False, tile_Z)[0]

            # kuma forward: z' = 1 - (1 - sigmoid(z)^a)^b, clipped
            t = tile_Z()
            nc.scalar.activation(t[:], z[:], AF.Exp, scale=-1.0)
            nc.scalar.activation(t[:], t[:], AF.Identity, bias=1.0)
            nc.vector.reciprocal(t[:], t[:])
            nc.vector.tensor_scalar(t[:], t[:], EPS, 1.0 - EPS,
                                    Alu.max, Alu.min)
            nc.scalar.activation(t[:], t[:], AF.Ln)
            nc.scalar.activation(t[:], t[:], AF.Exp, scale=a_ap)
            nc.scalar.activation(t[:], t[:], AF.Copy, bias=1.0, scale=-1.0)
            nc.vector.tensor_scalar(t[:], t[:], EPS, None, Alu.max)
            nc.scalar.activation(t[:], t[:], AF.Ln)
            nc.scalar.activation(t[:], t[:], AF.Exp, scale=b_ap)
            zp = tile_Z()
            nc.scalar.activation(zp[:], t[:], AF.Copy, bias=1.0, scale=-1.0)
            nc.vector.tensor_scalar(zp[:], zp[:], EPS, 1.0 - EPS,
                                    Alu.max, Alu.min)

            # residual VQ, 4 levels
            r = tile_Z()
            nc.vector.tensor_copy(r[:], zp[:])
            xq = tile_Z()
            for sc in range(N // P):
                scs = ts(sc, P)
                idx_t = vq.tile([P, 4], u32, tag="idxt", name="idxt")
                for l in range(4):
                    ps_s = pvs.tile([P, 256], f32, tag="ps_s", name="ps_s")
                    nc.tensor.matmul(ps_s[:], r[:, scs], cbt2[l][:],
                                     start=True, stop=True)
                    s_sb = vq.tile([P, 256], f32, tag="s_sb", name="s_sb")
                    nc.vector.tensor_tensor(s_sb[:], ps_s[:], nc2b[l][:],
                                            Alu.add)
                    mx8 = vq.tile([P, 8], f32, tag="mx8", name="mx8")
                    nc.vector.max(mx8[:], s_sb[:])
                    ix8 = vq.tile([P, 8], u32, tag="ix8", name="ix8")
                    nc.vector.max_index(ix8[:], mx8[:], s_sb[:])
                    nc.vector.tensor_copy(idx_t[:, l:l + 1], ix8[:, 0:1])
                    eq = vq.tile([P, 256], f32, tag="eq", name="eq")
                    nc.vector.tensor_scalar(eq[:], s_sb[:], mx8[:, 0:1],
                                            None, Alu.is_equal)
                    eqT = []
                    for j in range(2):
                        pst = pvt.tile([P, P], f32, tag="vt", name="vt")
                        nc.tensor.transpose(pst[:], eq[:, ts(j, P)], ident[:])
                        et = vq.tile([P, P], f32, tag=f"eqT{j}",
                                     name=f"eqT{j}")
                        nc.vector.tensor_copy(et[:], pst[:])
                        eqT.append(et)
                    ps_q = pvq.tile([P, P], f32, tag="ps_q", name="ps_q")
                    nc.tensor.matmul(ps_q[:], cbn0[l][:], eqT[0][:],
                                     start=True, stop=False)
                    nc.tensor.matmul(ps_q[:], cbn1[l][:], eqT[1][:],
                                     start=False, stop=True)
                    q_sb = vq.tile([P, P], f32, tag="q_sb", name="q_sb")
                    nc.vector.tensor_copy(q_sb[:], ps_q[:])
                    if l == 0:
                        nc.vector.tensor_copy(xq[:, scs], q_sb[:])
                    else:
                        nc.vector.tensor_add(xq[:, scs], xq[:, scs], q_sb[:])
                    nc.vector.tensor_sub(r[:, scs], r[:, scs], q_sb[:])
                    r2 = vq.tile([P, P], f32, tag="r2", name="r2")
                    nc.scalar.square(r2[:], r[:, scs])
                    red = vq.tile([P, 1], f32, tag="red", name="red")
                    nc.vector.tensor_reduce(red[:], r2[:], axis=AX.X,
                                            op=Alu.add)
                    nc.vector.tensor_add(sse[:, l:l + 1], sse[:, l:l + 1],
                                         red[:])
                nc.sync.dma_start(
                    d["indices"][row0 + sc * P: row0 + (sc + 1) * P, :],
                    idx_t[:].bitcast(i32))

            # kuma inverse on xq (z_q) and z' (z_recon); nvq loss
            zq = tile_Z(bf16)
            kuma_inv(xq, zq)
            zrec = tile_Z()
            kuma_inv(zp, zrec)
            dn = tile_Z()
            nc.vector.tensor_sub(dn[:], zrec[:], z[:])
            nc.scalar.square(dn[:], dn[:])
            redn = vq.tile([P, 1], f32, tag="red", name="red")
            nc.vector.tensor_reduce(redn[:], dn[:], axis=AX.X, op=Alu.add)
            nc.vector.tensor_add(sse[:, 4:5], sse[:, 4:5], redn[:])
            return zq

        def emit_back(c, zq):
            """decoder (tf32) + transpose-and-store."""
            row0 = c * N
            tile_Br = lambda: tile_B(bf16)
            tile_Ar = lambda: tile_A(bf16)
            g0 = mlp_layer([zq], "dw0", dbt[0], 128, 512, True, tile_Br,
                           tf32=True)
            g1 = mlp_layer(g0, "dw1", dbt[1], 512, 1024, True, tile_Ar,
                           tf32=True)
            g2 = mlp_layer(g1, "dw2", dbt[2], 1024, 2048, True, tile_Br,
                           tf32=True)
            om = mlp_layer(g2, "dw3", dbt[3], 2048, 768, False, tile_A,
                           tf32=True)

            for bt in range(N // P):
                ot = ost.tile([P, 768], f32, tag="ost", name="ost")
                for ftm in range(6):
                    pst = ppt.tile([P, P], f32, tag="pt", name="pt")
                    nc.tensor.transpose(pst[:], om[ftm][:, ts(bt, P)],
                                        ident[:])
                    nc.vector.tensor_copy(ot[:, ts(ftm, P)], pst[:])
                nc.sync.dma_start(
                    d["out"][row0 + bt * P: row0 + (bt + 1) * P, :], ot[:])

        zqs = [emit_front(c) for c in range(nchunks)]
        for c in range(nchunks):
            emit_back(c, zqs[c])

        nc.sync.dma_start(d["losses"][:], sse[:])


# --------------------------------------------------------------------------
# host side
# --------------------------------------------------------------------------

def _stripes(w):
    """[fin, fout] -> [fout//128, 128, fin] so device weight DMAs are
    contiguous and w_stripe[m][:, kt*128:(kt+1)*128] is the lhsT k-tile."""
    fin, fout = w.shape
    return np.ascontiguousarray(
        w.reshape(fin // P, P, fout // P, P).transpose(2, 1, 0, 3)
        .reshape(fout // P, P, fin))


def _bias_cols(b):
    return np.ascontiguousarray(b.reshape(-1, P).T)


def _tf32_round(w):
    """Round-to-nearest-even fp32 -> tf32 (10-bit mantissa) bit pattern."""
    u = w.view(np.uint32)
    r = (u + np.uint32(0xFFF) + ((u >> np.uint32(13)) & np.uint32(1)))
    return (r & np.uint32(0xFFFFE000)).view(np.float32)


def prep_inputs(inputs):
    inp = {k: np.ascontiguousarray(np.asarray(v)) for k, v in inputs.items()}
    cb = inp["codebooks"].astype(np.float32)
    shared = {}
    import ml_dtypes
    for i in range(4):
        w = inp[f"enc_w{i}"].astype(np.float32)
        if i < 2:  # split layers: bf16 hi/lo weight pairs
            wh = w.astype(ml_dtypes.bfloat16)
            wl = (w - wh.astype(np.float32)).astype(ml_dtypes.bfloat16)
            shared[f"ewh{i}"] = _stripes(wh)
            shared[f"ewl{i}"] = _stripes(wl)
        else:
            shared[f"ew{i}"] = _stripes(w)
        shared[f"eb{i}"] = _bias_cols(inp[f"enc_b{i}"].astype(np.float32))
        shared[f"dw{i}"] = _stripes(
            inp[f"dec_w{i}"].astype(np.float32).astype(ml_dtypes.bfloat16))
        shared[f"db{i}"] = _bias_cols(inp[f"dec_b{i}"].astype(np.float32))
    shared["cbT2"] = np.ascontiguousarray(
        (2.0 * cb).transpose(0, 2, 1)).astype(np.float32)
    shared["cbN"] = cb
    c2 = (cb * cb).sum(-1)  # [4, 256] fp32
    shared["nc2b"] = np.ascontiguousarray(
        np.broadcast_to((-c2)[:, None, :], (4, P, 256))).astype(np.float32)
    a_raw = inp["a_raw"].astype(np.float32)
    b_raw = inp["b_raw"].astype(np.float32)
    a = np.log1p(np.exp(a_raw)).astype(np.float32) + np.float32(EPS)
    b = np.log1p(np.exp(b_raw)).astype(np.float32) + np.float32(EPS)
    shared["abp"] = np.stack(
        [a, b, (np.float32(1.0) / a), (np.float32(1.0) / b)],
        axis=1).astype(np.float32)
    shared["ident"] = np.eye(P, dtype=np.float32)
    return inp["x"].astype(np.float32), shared


def combine_outputs(results, bc, btot):
    out = np.concatenate([r["out"] for r in results], axis=0)
    indices = np.concatenate([r["indices"] for r in results], axis=0)
    sse = np.zeros((P, 8), np.float64)
    for r in results:
        sse += r["losses"].astype(np.float64)
    nel = float(btot * 128)
    rq = np.mean([(1.0 + BETA) * sse[:, l].sum() / nel for l in range(4)])
    nvq = sse[:, 4].sum() / nel
    loss = np.float32(rq + nvq)
    return out, loss, indices.astype(np.int32)


def run(inputs, trace=False):
    from concourse.bass_utils import run_bass_kernel_spmd
    x, shared = prep_inputs(inputs)
    nc = build_program(BC)
    in_maps = []
    for c in range(NCORES):
        m = dict(shared)
        m["x"] = np.ascontiguousarray(x[c * BC:(c + 1) * BC])
        in_maps.append(m)
    res = run_bass_kernel_spmd(nc, in_maps, list(range(NCORES)), trace=trace)
    out, loss, indices = combine_outputs(res.results, BC, B_FULL)
    return (out, loss, indices), res, nc


def kernel(**inputs):
    (out, loss, indices), _, _ = run(inputs, trace=False)
    return out, loss, indices


# revision 33
# speedup vs baseline: 1.0120x; 1.0090x over previous
"""Trainium2 Bass kernel for nn_NURQVAE_15745350107776 (vq_codebook).

Pure data-parallel over 8 NeuronCores: batch dim of x sharded 2048 rows/core,
MLP weights + codebooks replicated. Per core everything runs feature-major
(features on partitions, batch on the free dim) so every matmul contracts over
the partition axis; x / out are transposed at the edges with PE transposes.

RVQ per level: score[b,k] = 2*r.c - ||c||^2 via one PE matmul + DVE add,
argmax via DVE max/max_index, one-hot gather back through the PE.
Encoder + VQ run in true fp32 (argmin near-ties rule out lower precision);
decoder matmuls run as tf32 (float32r) whose rounding noise is far below this
problem's fp32 argmin-tie envelope.

Emission is split into front(chunk)=enc+VQ+kuma and back(chunk)=dec+store so
the scheduler fills each chunk's VQ valley with the other chunk's PE work.
Loss partial sums are returned per-core and reduced on the host (the only
"collective" this problem needs).
"""

import sys
import numpy as np

try:
    import concourse  # noqa: F401
except ImportError:  # grading env fallback
    sys.path.insert(0, "/opt/trn_rl_repo")

import concourse.bass as bass
import concourse.mybir as mybir
import concourse.tile as tile
from concourse import bacc
from concourse.alu_op_type import AluOpType as Alu
from concourse.bass import ts

P = 128
B_FULL = 16384
NCORES = 8
BC = B_FULL // NCORES  # 2048 rows per core
NCHUNK_COLS = 512      # batch columns per chunk (feature-major free dim)
ENC = [768, 2048, 1024, 512, 128]
DEC = [128, 512, 1024, 2048, 768]
EPS = 1e-6
BETA = 0.25

f32 = mybir.dt.float32
f32r = mybir.dt.float32r
bf16 = mybir.dt.bfloat16
i32 = mybir.dt.int32
u32 = mybir.dt.uint32
AF = mybir.ActivationFunctionType
AX = mybir.AxisListType

_ONE_SET_ONLY = True  # keep exp+ln in natural_log_exp_and_others


def _patch_act_tables():
    """Hide redundant exp/ln-bearing table sets from the set chooser so every
    activation lands in natural_log_exp_and_others (which also has relu /
    copy / identity / square). Index-preserving: set ids stay aligned with
    act_info.json, only the advertised contents shrink."""
    from concourse import hw_specs
    if getattr(hw_specs, "_nurq_patched", False):
        return
    orig = hw_specs.get_activation_tables

    def patched(arch):
        tables = dict(orig(arch))
        keep = tables.get("natural_log_exp_and_others")
        if keep:
            for name in ("exp_and_others", "exp_and_friends", "natural_log"):
                if name in tables:
                    tables[name] = set()
        return tables

    hw_specs.get_activation_tables = patched
    hw_specs._nurq_patched = True
    import concourse.bacc as bacc_mod
    if hasattr(bacc_mod, "get_activation_tables"):
        bacc_mod.get_activation_tables = patched


# --------------------------------------------------------------------------
# device program
# --------------------------------------------------------------------------

def build_program(bc=BC):
    if _ONE_SET_ONLY:
        _patch_act_tables()
    nc = bacc.Bacc("TRN2", target_bir_lowering=False, debug=False,
                   num_devices=NCORES)
    d = {}
    d["x"] = nc.dram_tensor("x", [bc, 768], f32, kind="ExternalInput")
    for i in range(4):
        m = ENC[i + 1] // P
        if i < 2:
            d[f"ewh{i}"] = nc.dram_tensor(f"ewh{i}", [m, P, ENC[i]], bf16,
                                          kind="ExternalInput")
            d[f"ewl{i}"] = nc.dram_tensor(f"ewl{i}", [m, P, ENC[i]], bf16,
                                          kind="ExternalInput")
        else:
            d[f"ew{i}"] = nc.dram_tensor(f"ew{i}", [m, P, ENC[i]], f32,
                                         kind="ExternalInput")
        d[f"eb{i}"] = nc.dram_tensor(f"eb{i}", [P, m], f32,
                                     kind="ExternalInput")
        m = DEC[i + 1] // P
        d[f"dw{i}"] = nc.dram_tensor(f"dw{i}", [m, P, DEC[i]], bf16,
                                     kind="ExternalInput")
        d[f"db{i}"] = nc.dram_tensor(f"db{i}", [P, m], f32,
                                     kind="ExternalInput")
    d["cbT2"] = nc.dram_tensor("cbT2", [4, P, 256], f32, kind="ExternalInput")
    d["cbN"] = nc.dram_tensor("cbN", [4, 256, P], f32, kind="ExternalInput")
    d["nc2b"] = nc.dram_tensor("nc2b", [4, P, 256], f32, kind="ExternalInput")
    d["abp"] = nc.dram_tensor("abp", [P, 4], f32, kind="ExternalInput")
    d["ident"] = nc.dram_tensor("ident", [P, P], f32, kind="ExternalInput")

    d["out"] = nc.dram_tensor("out", [bc, 768], f32, kind="ExternalOutput")
    d["indices"] = nc.dram_tensor("indices", [bc, 4], i32,
                                  kind="ExternalOutput")
    d["losses"] = nc.dram_tensor("losses", [P, 8], f32, kind="ExternalOutput")

    with tile.TileContext(nc) as tc:
        _emit(tc, nc, d, bc)
    nc.compile()
    return nc


def _emit(tc, nc, d, bc):
    from contextlib import ExitStack
    N = NCHUNK_COLS
    nchunks = bc // N
    NH = N // 512

    ctx = ExitStack()
    with ctx:
        const = ctx.enter_context(tc.tile_pool(name="const", bufs=1))
        poolA = ctx.enter_context(tc.tile_pool(name="A", bufs=12))
        poolB = ctx.enter_context(tc.tile_pool(name="B", bufs=17))
        poolZ = ctx.enter_context(tc.tile_pool(name="Z", bufs=6))
        poolHE = ctx.enter_context(tc.tile_pool(name="HE", bufs=41))
        poolHF = ctx.enter_context(tc.tile_pool(name="HF", bufs=7))
        xst = ctx.enter_context(tc.tile_pool(name="xst", bufs=2))
        ost = ctx.enter_context(tc.tile_pool(name="ost", bufs=2))
        vq = ctx.enter_context(tc.tile_pool(name="vq", bufs=2))
        wpools = {
            fin: ctx.enter_context(tc.tile_pool(name=f"w{fin}", bufs=b))
            for fin, b in ((768, 2), (2048, 3), (1024, 2), (512, 2), (128, 2))
        }
        wbpools = {
            768: ctx.enter_context(tc.tile_pool(name="wb768", bufs=6)),
            2048: ctx.enter_context(tc.tile_pool(name="wb2048", bufs=4)),
        }
        PSUM = bass.MemorySpace.PSUM
        pmm = ctx.enter_context(tc.tile_pool(name="pmm", bufs=4, space=PSUM))
        ppt = ctx.enter_context(tc.tile_pool(name="ppt", bufs=1, space=PSUM))
        pvt = ctx.enter_context(tc.tile_pool(name="pvt", bufs=1, space=PSUM))
        pvs = ctx.enter_context(tc.tile_pool(name="pvs", bufs=1, space=PSUM))
        pvq = ctx.enter_context(tc.tile_pool(name="pvq", bufs=1, space=PSUM))

        # ---- persistent constants ----
        ident = const.tile([P, P], f32, tag="ident", name="ident")
        nc.sync.dma_start(ident[:], d["ident"][:])
        abp = const.tile([P, 4], f32, tag="abp", name="abp")
        nc.sync.dma_start(abp[:], d["abp"][:])
        a_ap, b_ap = abp[:, 0:1], abp[:, 1:2]
        ia_ap, ib_ap = abp[:, 2:3], abp[:, 3:4]
        cbt2, nc2b, cbn0, cbn1 = [], [], [], []
        for l in range(4):
            t = const.tile([P, 256], f32, tag=f"cbt2_{l}", name=f"cbt2_{l}")
            nc.sync.dma_start(t[:], d["cbT2"][l])
            cbt2.append(t)
            t = const.tile([P, 256], f32, tag=f"nc2b_{l}", name=f"nc2b_{l}")
            nc.sync.dma_start(t[:], d["nc2b"][l])
            nc2b.append(t)
            t = const.tile([P, P], f32, tag=f"cbn0_{l}", name=f"cbn0_{l}")
            nc.sync.dma_start(t[:], d["cbN"][l, 0:128, :])
            cbn0.append(t)
            t = const.tile([P, P], f32, tag=f"cbn1_{l}", name=f"cbn1_{l}")
            nc.sync.dma_start(t[:], d["cbN"][l, 128:256, :])
            cbn1.append(t)
        ebt, dbt = [], []
        for i in range(4):
            t = const.tile([P, ENC[i + 1] // P], f32, tag=f"eb{i}",
                           name=f"eb{i}")
            nc.sync.dma_start(t[:], d[f"eb{i}"][:])
            ebt.append(t)
            t = const.tile([P, DEC[i + 1] // P], f32, tag=f"db{i}",
                           name=f"db{i}")
            nc.sync.dma_start(t[:], d[f"db{i}"][:])
            dbt.append(t)
        sse = const.tile([P, 8], f32, tag="sse", name="sse")
        nc.vector.memset(sse[:], 0.0)

        def tile_A(dt=f32):
            return poolA.tile([P, N], dt, tag="A", name="A")

        def tile_B(dt=f32):
            return poolB.tile([P, N], dt, tag="B", name="B")

        def tile_Z(dt=f32):
            return poolZ.tile([P, N], dt, tag="Z", name="Z")

        def to_pair(src_tile):
            """f32 tile -> (hi, lo) bf16 pair; hi = bf16(x), lo = bf16(x-hi)."""
            hi = poolHE.tile([P, N], bf16, tag="HE", name="HE")
            nc.vector.tensor_copy(hi[:], src_tile[:])
            lo = poolHE.tile([P, N], bf16, tag="HE", name="HE")
            nc.vector.tensor_sub(lo[:], src_tile[:], hi[:])
            return (hi, lo)

        def mlp_layer(h_in, wname, bias, fin, fout, relu, out_alloc,
                      tf32=False):
            K, M = fin // P, fout // P
            outs = []
            wdt = bf16 if tf32 else f32
            for m in range(M):
                wt = wpools[fin].tile([P, fin], wdt, tag=f"w{fin}",
                                      name=f"w{fin}")
                nc.sync.dma_start(wt[:], d[wname][m])
                ps = pmm.tile([P, N], f32, tag="mm", name="mm")
                for kt in range(K):
                    lhsT = wt[:, ts(kt, P)]
                    for nh in range(NH):
                        rhs = h_in[kt][:, ts(nh, 512)]
                        nc.tensor.matmul(
                            ps[:, ts(nh, 512)], lhsT, rhs,
                            start=(kt == 0), stop=(kt == K - 1))
                ot = out_alloc()
                nc.scalar.activation(ot[:], ps[:],
                                     AF.Relu if relu else AF.Identity,
                                     bias=bias[:, m:m + 1])
                outs.append(ot)
            return outs

        def mlp_split_layer(h_pairs, i, bias, fin, fout, relu,
                            out_pairs, out_alloc=None):
            """3-term bf16 split: ps += Wh.hh + Wh.hl + Wl.hh.
            h_pairs: list of (hi, lo) bf16 tiles. Returns bf16 pairs
            (out_pairs=True, via transient HF tiles) or persistent f32
            tiles from out_alloc."""
            K, M = fin // P, fout // P
            outs = []
            for m in range(M):
                wh = wbpools[fin].tile([P, fin], bf16, tag=f"wb{fin}",
                                       name=f"wb{fin}")
                nc.sync.dma_start(wh[:], d[f"ewh{i}"][m])
                wl = wbpools[fin].tile([P, fin], bf16, tag=f"wb{fin}",
                                       name=f"wb{fin}")
                nc.sync.dma_start(wl[:], d[f"ewl{i}"][m])
                ps = pmm.tile([P, N], f32, tag="mm", name="mm")
                for kt in range(K):
                    hh, hl = h_pairs[kt]
                    terms = ((wh, hh), (wh, hl), (wl, hh))
                    for tix, (wmat, hmat) in enumerate(terms):
                        nc.tensor.matmul(
                            ps[:], wmat[:, ts(kt, P)], hmat[:],
                            start=(kt == 0 and tix == 0),
                            stop=(kt == K - 1 and tix == 2))
                if out_pairs:
                    hf = poolHF.tile([P, N], f32, tag="HF", name="HF")
                else:
                    hf = out_alloc()
                nc.scalar.activation(hf[:], ps[:],
                                     AF.Relu if relu else AF.Identity,
                                     bias=bias[:, m:m + 1])
                outs.append(to_pair(hf) if out_pairs else hf)
            return outs

        def kuma_inv(y_tile, out_tile):
            # out = logit(clip((1 - clip(1-y)^(1/b))^(1/a)))
            t = tile_Z()
            nc.vector.tensor_scalar(t[:], y_tile[:], EPS, 1.0 - EPS,
                                    Alu.max, Alu.min)
            nc.scalar.activation(t[:], t[:], AF.Copy, bias=1.0, scale=-1.0)
            nc.scalar.activation(t[:], t[:], AF.Ln)
            nc.scalar.activation(t[:], t[:], AF.Exp, scale=ib_ap)
            nc.scalar.activation(t[:], t[:], AF.Copy, bias=1.0, scale=-1.0)
            nc.vector.tensor_scalar(t[:], t[:], EPS, 1.0 - EPS,
                                    Alu.max, Alu.min)
            nc.scalar.activation(t[:], t[:], AF.Ln)
            nc.scalar.activation(t[:], t[:], AF.Exp, scale=ia_ap)
            nc.vector.tensor_scalar(t[:], t[:], EPS, 1.0 - EPS,
                                    Alu.max, Alu.min)
            u = tile_Z()
            nc.scalar.activation(u[:], t[:], AF.Copy, bias=1.0, scale=-1.0)
            nc.scalar.activation(u[:], u[:], AF.Ln)
            nc.scalar.activation(t[:], t[:], AF.Ln)
            nc.vector.tensor_sub(out_tile[:], t[:], u[:])

        def emit_front(c):
            """x load + transpose, encoder, kuma fwd, VQ, kuma inv.
            Returns the zq tile (decoder input)."""
            row0 = c * N

            h0f = [poolHF.tile([P, N], f32, tag="HF", name="HF")
                   for _ in range(6)]
            for bt in range(N // P):
                xa = xst.tile([P, 768], f32, tag="xst", name="xst")
                nc.sync.dma_start(
                    xa[:], d["x"][row0 + bt * P: row0 + (bt + 1) * P, :])
                for ft in range(6):
                    pst = ppt.tile([P, P], f32, tag="pt", name="pt")
                    nc.tensor.transpose(pst[:], xa[:, ts(ft, P)], ident[:])
                    nc.vector.tensor_copy(h0f[ft][:, ts(bt, P)], pst[:])
            h0 = [to_pair(t) for t in h0f]

            h1 = mlp_split_layer(h0, 0, ebt[0], 768, 2048, True, True)
            h2 = mlp_split_layer(h1, 1, ebt[1], 2048, 1024, True, False,
                                 tile_A)
            h3 = mlp_layer(h2, "ew2", ebt[2], 1024, 512, True, tile_B)
            z = mlp_layer(h3, "ew3", ebt[3], 512, 128, False, tile_Z)[0]

            # kuma forward: z' = 1 - (1 - sigmoid(z)^a)^b, clipped
            t = tile_Z()
            nc.scalar.activation(t[:], z[:], AF.Exp, scale=-1.0)
            nc.scalar.activation(t[:], t[:], AF.Identity, bias=1.0)
            nc.vector.reciprocal(t[:], t[:])
            nc.vector.tensor_scalar(t[:], t[:], EPS, 1.0 - EPS,
                                    Alu.max, Alu.min)
            nc.scalar.activation(t[:], t[:], AF.Ln)
            nc.scalar.activation(t[:], t[:], AF.Exp, scale=a_ap)
            nc.scalar.activation(t[:], t[:], AF.Copy, bias=1.0, scale=-1.0)
            nc.vector.tensor_scalar(t[:], t[:], EPS, None, Alu.max)
            nc.scalar.activation(t[:], t[:], AF.Ln)
            nc.scalar.activation(t[:], t[:], AF.Exp, scale=b_ap)
            zp = tile_Z()
            nc.scalar.activation(zp[:], t[:], AF.Copy, bias=1.0, scale=-1.0)
            nc.vector.tensor_scalar(zp[:], zp[:], EPS, 1.0 - EPS,
                                    Alu.max, Alu.min)

            # residual VQ, 4 levels
            r = tile_Z()
            nc.vector.tensor_copy(r[:], zp[:])
            xq = tile_Z()
            for sc in range(N // P):
                scs = ts(sc, P)
                idx_t = vq.tile([P, 4], u32, tag="idxt", name="idxt")
                for l in range(4):
                    ps_s = pvs.tile([P, 256], f32, tag="ps_s", name="ps_s")
                    nc.tensor.matmul(ps_s[:], r[:, scs], cbt2[l][:],
                                     start=True, stop=True)
                    s_sb = vq.tile([P, 256], f32, tag="s_sb", name="s_sb")
                    nc.vector.tensor_tensor(s_sb[:], ps_s[:], nc2b[l][:],
                                            Alu.add)
                    mx8 = vq.tile([P, 8], f32, tag="mx8", name="mx8")
                    nc.vector.max(mx8[:], s_sb[:])
                    ix8 = vq.tile([P, 8], u32, tag="ix8", name="ix8")
                    nc.vector.max_index(ix8[:], mx8[:], s_sb[:])
                    nc.vector.tensor_copy(idx_t[:, l:l + 1], ix8[:, 0:1])
                    eq = vq.tile([P, 256], f32, tag="eq", name="eq")
                    nc.vector.tensor_scalar(eq[:], s_sb[:], mx8[:, 0:1],
                                            None, Alu.is_equal)
                    eqT = []
                    for j in range(2):
                        pst = pvt.tile([P, P], f32, tag="vt", name="vt")
                        nc.tensor.transpose(pst[:], eq[:, ts(j, P)], ident[:])
                        et = vq.tile([P, P], f32, tag=f"eqT{j}",
                                     name=f"eqT{j}")
                        nc.vector.tensor_copy(et[:], pst[:])
                        eqT.append(et)
                    ps_q = pvq.tile([P, P], f32, tag="ps_q", name="ps_q")
                    nc.tensor.matmul(ps_q[:], cbn0[l][:], eqT[0][:],
                                     start=True, stop=False)
                    nc.tensor.matmul(ps_q[:], cbn1[l][:], eqT[1][:],
                                     start=False, stop=True)
                    q_sb = vq.tile([P, P], f32, tag="q_sb", name="q_sb")
                    nc.vector.tensor_copy(q_sb[:], ps_q[:])
                    if l == 0:
                        nc.vector.tensor_copy(xq[:, scs], q_sb[:])
                    else:
                        nc.vector.tensor_add(xq[:, scs], xq[:, scs], q_sb[:])
                    nc.vector.tensor_sub(r[:, scs], r[:, scs], q_sb[:])
                    r2 = vq.tile([P, P], f32, tag="r2", name="r2")
                    nc.scalar.square(r2[:], r[:, scs])
                    red = vq.tile([P, 1], f32, tag="red", name="red")
                    nc.vector.tensor_reduce(red[:], r2[:], axis=AX.X,
                                            op=Alu.add)
                    nc.vector.tensor_add(sse[:, l:l + 1], sse[:, l:l + 1],
                                         red[:])
                nc.sync.dma_start(
                    d["indices"][row0 + sc * P: row0 + (sc + 1) * P, :],
                    idx_t[:].bitcast(i32))

            # kuma inverse on xq (z_q) and z' (z_recon); nvq loss
            zq = tile_Z(bf16)
            kuma_inv(xq, zq)
            zrec = tile_Z()
            kuma_inv(zp, zrec)
            dn = tile_Z()
            nc.vector.tensor_sub(dn[:], zrec[:], z[:])
            nc.scalar.square(dn[:], dn[:])
            redn = vq.tile([P, 1], f32, tag="red", name="red")
            nc.vector.tensor_reduce(redn[:], dn[:], axis=AX.X, op=Alu.add)
            nc.vector.tensor_add(sse[:, 4:5], sse[:, 4:5], redn[:])
            return zq

        def emit_back(c, zq):
            """decoder (tf32) + transpose-and-store."""
            row0 = c * N
            tile_Br = lambda: tile_B(bf16)
            tile_Ar = lambda: tile_A(bf16)
            g0 = mlp_layer([zq], "dw0", dbt[0], 128, 512, True, tile_Br,
                           tf32=True)
            g1 = mlp_layer(g0, "dw1", dbt[1], 512, 1024, True, tile_Ar,
                           tf32=True)
            g2 = mlp_layer(g1, "dw2", dbt[2], 1024, 2048, True, tile_Br,
                           tf32=True)
            om = mlp_layer(g2, "dw3", dbt[3], 2048, 768, False, tile_A,
                           tf32=True)

            for bt in range(N // P):
                ot = ost.tile([P, 768], f32, tag="ost", name="ost")
                for ftm in range(6):
                    pst = ppt.tile([P, P], f32, tag="pt", name="pt")
                    nc.tensor.transpose(pst[:], om[ftm][:, ts(bt, P)],
                                        ident[:])
                    nc.vector.tensor_copy(ot[:, ts(ftm, P)], pst[:])
                nc.sync.dma_start(
                    d["out"][row0 + bt * P: row0 + (bt + 1) * P, :], ot[:])

        zqs = [emit_front(c) for c in range(nchunks)]
        for c in range(nchunks):
            emit_back(c, zqs[c])

        nc.sync.dma_start(d["losses"][:], sse[:])


# --------------------------------------------------------------------------
# host side
# --------------------------------------------------------------------------

def _stripes(w):
    """[fin, fout] -> [fout//128, 128, fin] so device weight DMAs are
    contiguous and w_stripe[m][:, kt*128:(kt+1)*128] is the lhsT k-tile."""
    fin, fout = w.shape
    return np.ascontiguousarray(
        w.reshape(fin // P, P, fout // P, P).transpose(2, 1, 0, 3)
        .reshape(fout // P, P, fin))


def _bias_cols(b):
    return np.ascontiguousarray(b.reshape(-1, P).T)


def _tf32_round(w):
    """Round-to-nearest-even fp32 -> tf32 (10-bit mantissa) bit pattern."""
    u = w.view(np.uint32)
    r = (u + np.uint32(0xFFF) + ((u >> np.uint32(13)) & np.uint32(1)))
    return (r & np.uint32(0xFFFFE000)).view(np.float32)


def prep_inputs(inputs):
    inp = {k: np.ascontiguousarray(np.asarray(v)) for k, v in inputs.items()}
    cb = inp["codebooks"].astype(np.float32)
    shared = {}
    import ml_dtypes
    for i in range(4):
        w = inp[f"enc_w{i}"].astype(np.float32)
        if i < 2:  # split layers: bf16 hi/lo weight pairs
            wh = w.astype(ml_dtypes.bfloat16)
            wl = (w - wh.astype(np.float32)).astype(ml_dtypes.bfloat16)
            shared[f"ewh{i}"] = _stripes(wh)
            shared[f"ewl{i}"] = _stripes(wl)
        else:
            shared[f"ew{i}"] = _stripes(w)
        shared[f"eb{i}"] = _bias_cols(inp[f"enc_b{i}"].astype(np.float32))
        shared[f"dw{i}"] = _stripes(
            inp[f"dec_w{i}"].astype(np.float32).astype(ml_dtypes.bfloat16))
        shared[f"db{i}"] = _bias_cols(inp[f"dec_b{i}"].astype(np.float32))
    shared["cbT2"] = np.ascontiguousarray(
        (2.0 * cb).transpose(0, 2, 1)).astype(np.float32)
    shared["cbN"] = cb
    c2 = (cb * cb).sum(-1)  # [4, 256] fp32
    shared["nc2b"] = np.ascontiguousarray(
        np.broadcast_to((-c2)[:, None, :], (4, P, 256))).astype(np.float32)
    a_raw = inp["a_raw"].astype(np.float32)
    b_raw = inp["b_raw"].astype(np.float32)
    a = np.log1p(np.exp(a_raw)).astype(np.float32) + np.float32(EPS)
    b = np.log1p(np.exp(b_raw)).astype(np.float32) + np.float32(EPS)
    shared["abp"] = np.stack(
        [a, b, (np.float32(1.0) / a), (np.float32(1.0) / b)],
        axis=1).astype(np.float32)
    shared["ident"] = np.eye(P, dtype=np.float32)
    return inp["x"].astype(np.float32), shared


def combine_outputs(results, bc, btot):
    out = np.concatenate([r["out"] for r in results], axis=0)
    indices = np.concatenate([r["indices"] for r in results], axis=0)
    sse = np.zeros((P, 8), np.float64)
    for r in results:
        sse += r["losses"].astype(np.float64)
    nel = float(btot * 128)
    rq = np.mean([(1.0 + BETA) * sse[:, l].sum() / nel for l in range(4)])
    nvq = sse[:, 4].sum() / nel
    loss = np.float32(rq + nvq)
    return out, loss, indices.astype(np.int32)


def run(inputs, trace=False):
    from concourse.bass_utils import run_bass_kernel_spmd
    x, shared = prep_inputs(inputs)
    nc = build_program(BC)
    in_maps = []
    for c in range(NCORES):
        m = dict(shared)
        m["x"] = np.ascontiguousarray(x[c * BC:(c + 1) * BC])
        in_maps.append(m)
    res = run_bass_kernel_spmd(nc, in_maps, list(range(NCORES)), trace=trace)
    out, loss, indices = combine_outputs(res.results, BC, B_FULL)
    return (out, loss, indices), res, nc


def kernel(**inputs):
    (out, loss, indices), _, _ = run(inputs, trace=False)
    return out, loss, indices


# revision 34
# speedup vs baseline: 1.0297x; 1.0174x over previous
"""Trainium2 Bass kernel for nn_NURQVAE_15745350107776 (vq_codebook).

Pure data-parallel over 8 NeuronCores: batch dim of x sharded 2048 rows/core,
MLP weights + codebooks replicated. Per core everything runs feature-major
(features on partitions, batch on the free dim) so every matmul contracts over
the partition axis; x / out are transposed at the edges with PE transposes.

RVQ per level: score[b,k] = 2*r.c - ||c||^2 via one PE matmul + DVE add,
argmax via DVE max/max_index, one-hot gather back through the PE.
Encoder + VQ run in true fp32 (argmin near-ties rule out lower precision);
decoder matmuls run as tf32 (float32r) whose rounding noise is far below this
problem's fp32 argmin-tie envelope.

Emission is split into front(chunk)=enc+VQ+kuma and back(chunk)=dec+store so
the scheduler fills each chunk's VQ valley with the other chunk's PE work.
Loss partial sums are returned per-core and reduced on the host (the only
"collective" this problem needs).
"""

import sys
import numpy as np

try:
    import concourse  # noqa: F401
except ImportError:  # grading env fallback
    sys.path.insert(0, "/opt/trn_rl_repo")

import concourse.bass as bass
import concourse.mybir as mybir
import concourse.tile as tile
from concourse import bacc
from concourse.alu_op_type import AluOpType as Alu
from concourse.bass import ts

P = 128
B_FULL = 16384
NCORES = 8
BC = B_FULL // NCORES  # 2048 rows per core
NCHUNK_COLS = 512      # batch columns per chunk (feature-major free dim)
ENC = [768, 2048, 1024, 512, 128]
DEC = [128, 512, 1024, 2048, 768]
EPS = 1e-6
BETA = 0.25

f32 = mybir.dt.float32
f32r = mybir.dt.float32r
bf16 = mybir.dt.bfloat16
i32 = mybir.dt.int32
u32 = mybir.dt.uint32
AF = mybir.ActivationFunctionType
AX = mybir.AxisListType

_ONE_SET_ONLY = True  # keep exp+ln in natural_log_exp_and_others


def _patch_act_tables():
    """Hide redundant exp/ln-bearing table sets from the set chooser so every
    activation lands in natural_log_exp_and_others (which also has relu /
    copy / identity / square). Index-preserving: set ids stay aligned with
    act_info.json, only the advertised contents shrink."""
    from concourse import hw_specs
    if getattr(hw_specs, "_nurq_patched", False):
        return
    orig = hw_specs.get_activation_tables

    def patched(arch):
        tables = dict(orig(arch))
        keep = tables.get("natural_log_exp_and_others")
        if keep:
            for name in ("exp_and_others", "exp_and_friends", "natural_log"):
                if name in tables:
                    tables[name] = set()
        return tables

    hw_specs.get_activation_tables = patched
    hw_specs._nurq_patched = True
    import concourse.bacc as bacc_mod
    if hasattr(bacc_mod, "get_activation_tables"):
        bacc_mod.get_activation_tables = patched


# --------------------------------------------------------------------------
# device program
# --------------------------------------------------------------------------

def build_program(bc=BC):
    if _ONE_SET_ONLY:
        _patch_act_tables()
    nc = bacc.Bacc("TRN2", target_bir_lowering=False, debug=False,
                   num_devices=NCORES)
    d = {}
    d["x"] = nc.dram_tensor("x", [bc, 768], f32, kind="ExternalInput")
    for i in range(4):
        m = ENC[i + 1] // P
        if i < 2:
            d[f"ewh{i}"] = nc.dram_tensor(f"ewh{i}", [m, P, ENC[i]], bf16,
                                          kind="ExternalInput")
            d[f"ewl{i}"] = nc.dram_tensor(f"ewl{i}", [m, P, ENC[i]], bf16,
                                          kind="ExternalInput")
        else:
            d[f"ew{i}"] = nc.dram_tensor(f"ew{i}", [m, P, ENC[i]], f32,
                                         kind="ExternalInput")
        d[f"eb{i}"] = nc.dram_tensor(f"eb{i}", [P, m], f32,
                                     kind="ExternalInput")
        m = DEC[i + 1] // P
        d[f"dw{i}"] = nc.dram_tensor(f"dw{i}", [m, P, DEC[i]], bf16,
                                     kind="ExternalInput")
        d[f"db{i}"] = nc.dram_tensor(f"db{i}", [P, m], f32,
                                     kind="ExternalInput")
    d["cbT2"] = nc.dram_tensor("cbT2", [4, P, 256], f32, kind="ExternalInput")
    d["cbN"] = nc.dram_tensor("cbN", [4, 256, P], f32, kind="ExternalInput")
    d["nc2b"] = nc.dram_tensor("nc2b", [4, P, 256], f32, kind="ExternalInput")
    d["abp"] = nc.dram_tensor("abp", [P, 4], f32, kind="ExternalInput")
    d["ident"] = nc.dram_tensor("ident", [P, P], f32, kind="ExternalInput")

    d["out"] = nc.dram_tensor("out", [bc, 768], f32, kind="ExternalOutput")
    d["indices"] = nc.dram_tensor("indices", [bc, 4], i32,
                                  kind="ExternalOutput")
    d["losses"] = nc.dram_tensor("losses", [P, 8], f32, kind="ExternalOutput")

    with tile.TileContext(nc) as tc:
        _emit(tc, nc, d, bc)
    nc.compile()
    return nc


def _emit(tc, nc, d, bc):
    from contextlib import ExitStack
    N = NCHUNK_COLS
    nchunks = bc // N
    NH = N // 512

    ctx = ExitStack()
    with ctx:
        const = ctx.enter_context(tc.tile_pool(name="const", bufs=1))
        poolA = ctx.enter_context(tc.tile_pool(name="A", bufs=12))
        poolB = ctx.enter_context(tc.tile_pool(name="B", bufs=17))
        poolZ = ctx.enter_context(tc.tile_pool(name="Z", bufs=6))
        poolHE = ctx.enter_context(tc.tile_pool(name="HE", bufs=41))
        poolHF = ctx.enter_context(tc.tile_pool(name="HF", bufs=7))
        xst = ctx.enter_context(tc.tile_pool(name="xst", bufs=2))
        ost = ctx.enter_context(tc.tile_pool(name="ost", bufs=2))
        vq = ctx.enter_context(tc.tile_pool(name="vq", bufs=2))
        wpools = {
            fin: ctx.enter_context(tc.tile_pool(name=f"w{fin}", bufs=b))
            for fin, b in ((768, 2), (2048, 3), (1024, 2), (512, 2), (128, 2))
        }
        wbpools = {
            768: ctx.enter_context(tc.tile_pool(name="wb768", bufs=6)),
            2048: ctx.enter_context(tc.tile_pool(name="wb2048", bufs=4)),
        }
        PSUM = bass.MemorySpace.PSUM
        pmm = ctx.enter_context(tc.tile_pool(name="pmm", bufs=3, space=PSUM))
        ppt = ctx.enter_context(tc.tile_pool(name="ppt", bufs=2, space=PSUM))
        pvt = ctx.enter_context(tc.tile_pool(name="pvt", bufs=1, space=PSUM))
        pvs = ctx.enter_context(tc.tile_pool(name="pvs", bufs=1, space=PSUM))
        pvq = ctx.enter_context(tc.tile_pool(name="pvq", bufs=1, space=PSUM))

        # ---- persistent constants ----
        ident = const.tile([P, P], f32, tag="ident", name="ident")
        nc.sync.dma_start(ident[:], d["ident"][:])
        abp = const.tile([P, 4], f32, tag="abp", name="abp")
        nc.sync.dma_start(abp[:], d["abp"][:])
        a_ap, b_ap = abp[:, 0:1], abp[:, 1:2]
        ia_ap, ib_ap = abp[:, 2:3], abp[:, 3:4]
        cbt2, nc2b, cbn0, cbn1 = [], [], [], []
        for l in range(4):
            t = const.tile([P, 256], f32, tag=f"cbt2_{l}", name=f"cbt2_{l}")
            nc.sync.dma_start(t[:], d["cbT2"][l])
            cbt2.append(t)
            t = const.tile([P, 256], f32, tag=f"nc2b_{l}", name=f"nc2b_{l}")
            nc.sync.dma_start(t[:], d["nc2b"][l])
            nc2b.append(t)
            t = const.tile([P, P], f32, tag=f"cbn0_{l}", name=f"cbn0_{l}")
            nc.sync.dma_start(t[:], d["cbN"][l, 0:128, :])
            cbn0.append(t)
            t = const.tile([P, P], f32, tag=f"cbn1_{l}", name=f"cbn1_{l}")
            nc.sync.dma_start(t[:], d["cbN"][l, 128:256, :])
            cbn1.append(t)
        ebt, dbt = [], []
        for i in range(4):
            t = const.tile([P, ENC[i + 1] // P], f32, tag=f"eb{i}",
                           name=f"eb{i}")
            nc.sync.dma_start(t[:], d[f"eb{i}"][:])
            ebt.append(t)
            t = const.tile([P, DEC[i + 1] // P], f32, tag=f"db{i}",
                           name=f"db{i}")
            nc.sync.dma_start(t[:], d[f"db{i}"][:])
            dbt.append(t)
        sse = const.tile([P, 8], f32, tag="sse", name="sse")
        nc.vector.memset(sse[:], 0.0)

        def tile_A(dt=f32):
            return poolA.tile([P, N], dt, tag="A", name="A")

        def tile_B(dt=f32):
            return poolB.tile([P, N], dt, tag="B", name="B")

        def tile_Z(dt=f32):
            return poolZ.tile([P, N], dt, tag="Z", name="Z")

        def to_pair(src_tile):
            """f32 tile -> (hi, lo) bf16 pair; hi = bf16(x), lo = bf16(x-hi)."""
            hi = poolHE.tile([P, N], bf16, tag="HE", name="HE")
            nc.vector.tensor_copy(hi[:], src_tile[:])
            lo = poolHE.tile([P, N], bf16, tag="HE", name="HE")
            nc.vector.tensor_sub(lo[:], src_tile[:], hi[:])
            return (hi, lo)

        def mlp_layer(h_in, wname, bias, fin, fout, relu, out_alloc,
                      tf32=False):
            K, M = fin // P, fout // P
            outs = []
            wdt = bf16 if tf32 else f32
            for m in range(M):
                wt = wpools[fin].tile([P, fin], wdt, tag=f"w{fin}",
                                      name=f"w{fin}")
                nc.sync.dma_start(wt[:], d[wname][m])
                ps = pmm.tile([P, N], f32, tag="mm", name="mm")
                for kt in range(K):
                    lhsT = wt[:, ts(kt, P)]
                    for nh in range(NH):
                        rhs = h_in[kt][:, ts(nh, 512)]
                        nc.tensor.matmul(
                            ps[:, ts(nh, 512)], lhsT, rhs,
                            start=(kt == 0), stop=(kt == K - 1))
                ot = out_alloc()
                nc.scalar.activation(ot[:], ps[:],
                                     AF.Relu if relu else AF.Identity,
                                     bias=bias[:, m:m + 1])
                outs.append(ot)
            return outs

        def mlp_split_layer(h_pairs, i, bias, fin, fout, relu,
                            out_pairs, out_alloc=None):
            """3-term bf16 split: ps += Wh.hh + Wh.hl + Wl.hh.
            h_pairs: list of (hi, lo) bf16 tiles. Returns bf16 pairs
            (out_pairs=True, via transient HF tiles) or persistent f32
            tiles from out_alloc."""
            K, M = fin // P, fout // P
            outs = []
            for m in range(M):
                wh = wbpools[fin].tile([P, fin], bf16, tag=f"wb{fin}",
                                       name=f"wb{fin}")
                nc.sync.dma_start(wh[:], d[f"ewh{i}"][m])
                wl = wbpools[fin].tile([P, fin], bf16, tag=f"wb{fin}",
                                       name=f"wb{fin}")
                nc.sync.dma_start(wl[:], d[f"ewl{i}"][m])
                ps = pmm.tile([P, N], f32, tag="mm", name="mm")
                for kt in range(K):
                    hh, hl = h_pairs[kt]
                    terms = ((wh, hh), (wh, hl), (wl, hh))
                    for tix, (wmat, hmat) in enumerate(terms):
                        nc.tensor.matmul(
                            ps[:], wmat[:, ts(kt, P)], hmat[:],
                            start=(kt == 0 and tix == 0),
                            stop=(kt == K - 1 and tix == 2))
                if out_pairs:
                    hf = poolHF.tile([P, N], f32, tag="HF", name="HF")
                else:
                    hf = out_alloc()
                nc.scalar.activation(hf[:], ps[:],
                                     AF.Relu if relu else AF.Identity,
                                     bias=bias[:, m:m + 1])
                outs.append(to_pair(hf) if out_pairs else hf)
            return outs

        def kuma_inv(y_tile, out_tile):
            # out = logit(clip((1 - clip(1-y)^(1/b))^(1/a)))
            t = tile_Z()
            nc.vector.tensor_scalar(t[:], y_tile[:], EPS, 1.0 - EPS,
                                    Alu.max, Alu.min)
            nc.scalar.activation(t[:], t[:], AF.Copy, bias=1.0, scale=-1.0)
            nc.scalar.activation(t[:], t[:], AF.Ln)
            nc.scalar.activation(t[:], t[:], AF.Exp, scale=ib_ap)
            nc.scalar.activation(t[:], t[:], AF.Copy, bias=1.0, scale=-1.0)
            nc.vector.tensor_scalar(t[:], t[:], EPS, 1.0 - EPS,
                                    Alu.max, Alu.min)
            nc.scalar.activation(t[:], t[:], AF.Ln)
            nc.scalar.activation(t[:], t[:], AF.Exp, scale=ia_ap)
            nc.vector.tensor_scalar(t[:], t[:], EPS, 1.0 - EPS,
                                    Alu.max, Alu.min)
            u = tile_Z()
            nc.scalar.activation(u[:], t[:], AF.Copy, bias=1.0, scale=-1.0)
            nc.scalar.activation(u[:], u[:], AF.Ln)
            nc.scalar.activation(t[:], t[:], AF.Ln)
            nc.vector.tensor_sub(out_tile[:], t[:], u[:])

        def emit_front(c):
            """x load + transpose, encoder, kuma fwd, VQ, kuma inv.
            Returns the zq tile (decoder input)."""
            row0 = c * N

            h0f = [poolHF.tile([P, N], f32, tag="HF", name="HF")
                   for _ in range(6)]
            for bt in range(N // P):
                xa = xst.tile([P, 768], f32, tag="xst", name="xst")
                nc.sync.dma_start(
                    xa[:], d["x"][row0 + bt * P: row0 + (bt + 1) * P, :])
                for ft in range(6):
                    pst = ppt.tile([P, P], f32, tag="pt", name="pt")
                    nc.tensor.transpose(pst[:], xa[:, ts(ft, P)], ident[:])
                    nc.vector.tensor_copy(h0f[ft][:, ts(bt, P)], pst[:])
            h0 = [to_pair(t) for t in h0f]

            h1 = mlp_split_layer(h0, 0, ebt[0], 768, 2048, True, True)
            h2 = mlp_split_layer(h1, 1, ebt[1], 2048, 1024, True, False,
                                 tile_A)
            h3 = mlp_layer(h2, "ew2", ebt[2], 1024, 512, True, tile_B)
            z = mlp_layer(h3, "ew3", ebt[3], 512, 128, False, tile_Z)[0]

            # kuma forward: z' = 1 - (1 - sigmoid(z)^a)^b, clipped
            t = tile_Z()
            nc.scalar.activation(t[:], z[:], AF.Exp, scale=-1.0)
            nc.scalar.activation(t[:], t[:], AF.Identity, bias=1.0)
            nc.vector.reciprocal(t[:], t[:])
            nc.vector.tensor_scalar(t[:], t[:], EPS, 1.0 - EPS,
                                    Alu.max, Alu.min)
            nc.scalar.activation(t[:], t[:], AF.Ln)
            nc.scalar.activation(t[:], t[:], AF.Exp, scale=a_ap)
            nc.scalar.activation(t[:], t[:], AF.Copy, bias=1.0, scale=-1.0)
            nc.vector.tensor_scalar(t[:], t[:], EPS, None, Alu.max)
            nc.scalar.activation(t[:], t[:], AF.Ln)
            nc.scalar.activation(t[:], t[:], AF.Exp, scale=b_ap)
            zp = tile_Z()
            nc.scalar.activation(zp[:], t[:], AF.Copy, bias=1.0, scale=-1.0)
            nc.vector.tensor_scalar(zp[:], zp[:], EPS, 1.0 - EPS,
                                    Alu.max, Alu.min)

            # residual VQ, 4 levels
            r = tile_Z()
            nc.vector.tensor_copy(r[:], zp[:])
            xq = tile_Z()
            for sc in range(N // P):
                scs = ts(sc, P)
                idx_t = vq.tile([P, 4], u32, tag="idxt", name="idxt")
                for l in range(4):
                    ps_s = pvs.tile([P, 256], f32, tag="ps_s", name="ps_s")
                    nc.tensor.matmul(ps_s[:], r[:, scs], cbt2[l][:],
                                     start=True, stop=True)
                    s_sb = vq.tile([P, 256], f32, tag="s_sb", name="s_sb")
                    nc.vector.tensor_tensor(s_sb[:], ps_s[:], nc2b[l][:],
                                            Alu.add)
                    mx8 = vq.tile([P, 8], f32, tag="mx8", name="mx8")
                    nc.vector.max(mx8[:], s_sb[:])
                    ix8 = vq.tile([P, 8], u32, tag="ix8", name="ix8")
                    nc.vector.max_index(ix8[:], mx8[:], s_sb[:])
                    nc.vector.tensor_copy(idx_t[:, l:l + 1], ix8[:, 0:1])
                    eq = vq.tile([P, 256], f32, tag="eq", name="eq")
                    nc.vector.tensor_scalar(eq[:], s_sb[:], mx8[:, 0:1],
                                            None, Alu.is_equal)
                    eqT = []
                    for j in range(2):
                        pst = pvt.tile([P, P], f32, tag="vt", name="vt")
                        nc.tensor.transpose(pst[:], eq[:, ts(j, P)], ident[:])
                        et = vq.tile([P, P], f32, tag=f"eqT{j}",
                                     name=f"eqT{j}")
                        nc.vector.tensor_copy(et[:], pst[:])
                        eqT.append(et)
                    ps_q = pvq.tile([P, P], f32, tag="ps_q", name="ps_q")
                    nc.tensor.matmul(ps_q[:], cbn0[l][:], eqT[0][:],
                                     start=True, stop=False)
                    nc.tensor.matmul(ps_q[:], cbn1[l][:], eqT[1][:],
                                     start=False, stop=True)
                    q_sb = vq.tile([P, P], f32, tag="q_sb", name="q_sb")
                    nc.vector.tensor_copy(q_sb[:], ps_q[:])
                    if l == 0:
                        nc.vector.tensor_copy(xq[:, scs], q_sb[:])
                    else:
                        nc.vector.tensor_add(xq[:, scs], xq[:, scs], q_sb[:])
                    nc.vector.tensor_sub(r[:, scs], r[:, scs], q_sb[:])
                    r2 = vq.tile([P, P], f32, tag="r2", name="r2")
                    nc.scalar.square(r2[:], r[:, scs])
                    red = vq.tile([P, 1], f32, tag="red", name="red")
                    nc.vector.tensor_reduce(red[:], r2[:], axis=AX.X,
                                            op=Alu.add)
                    nc.vector.tensor_add(sse[:, l:l + 1], sse[:, l:l + 1],
                                         red[:])
                nc.sync.dma_start(
                    d["indices"][row0 + sc * P: row0 + (sc + 1) * P, :],
                    idx_t[:].bitcast(i32))

            # kuma inverse on xq (z_q) and z' (z_recon); nvq loss
            zq = tile_Z(bf16)
            kuma_inv(xq, zq)
            zrec = tile_Z()
            kuma_inv(zp, zrec)
            dn = tile_Z()
            nc.vector.tensor_sub(dn[:], zrec[:], z[:])
            nc.scalar.square(dn[:], dn[:])
            redn = vq.tile([P, 1], f32, tag="red", name="red")
            nc.vector.tensor_reduce(redn[:], dn[:], axis=AX.X, op=Alu.add)
            nc.vector.tensor_add(sse[:, 4:5], sse[:, 4:5], redn[:])
            return zq

        def emit_back(c, zq):
            """decoder (tf32) + transpose-and-store."""
            row0 = c * N
            tile_Br = lambda: tile_B(bf16)
            tile_Ar = lambda: tile_A(bf16)
            g0 = mlp_layer([zq], "dw0", dbt[0], 128, 512, True, tile_Br,
                           tf32=True)
            g1 = mlp_layer(g0, "dw1", dbt[1], 512, 1024, True, tile_Ar,
                           tf32=True)
            g2 = mlp_layer(g1, "dw2", dbt[2], 1024, 2048, True, tile_Br,
                           tf32=True)
            om = mlp_layer(g2, "dw3", dbt[3], 2048, 768, False, tile_A,
                           tf32=True)

            for bt in range(N // P):
                ot = ost.tile([P, 768], f32, tag="ost", name="ost")
                for ftm in range(6):
                    pst = ppt.tile([P, P], f32, tag="pt", name="pt")
                    nc.tensor.transpose(pst[:], om[ftm][:, ts(bt, P)],
                                        ident[:])
                    nc.vector.tensor_copy(ot[:, ts(ftm, P)], pst[:])
                nc.sync.dma_start(
                    d["out"][row0 + bt * P: row0 + (bt + 1) * P, :], ot[:])

        zqs = [emit_front(c) for c in range(nchunks)]
        for c in range(nchunks):
            emit_back(c, zqs[c])

        nc.sync.dma_start(d["losses"][:], sse[:])


# --------------------------------------------------------------------------
# host side
# --------------------------------------------------------------------------

def _stripes(w):
    """[fin, fout] -> [fout//128, 128, fin] so device weight DMAs are
    contiguous and w_stripe[m][:, kt*128:(kt+1)*128] is the lhsT k-tile."""
    fin, fout = w.shape
    return np.ascontiguousarray(
        w.reshape(fin // P, P, fout // P, P).transpose(2, 1, 0, 3)
        .reshape(fout // P, P, fin))


def _bias_cols(b):
    return np.ascontiguousarray(b.reshape(-1, P).T)


def _tf32_round(w):
    """Round-to-nearest-even fp32 -> tf32 (10-bit mantissa) bit pattern."""
    u = w.view(np.uint32)
    r = (u + np.uint32(0xFFF) + ((u >> np.uint32(13)) & np.uint32(1)))
    return (r & np.uint32(0xFFFFE000)).view(np.float32)


def prep_inputs(inputs):
    inp = {k: np.ascontiguousarray(np.asarray(v)) for k, v in inputs.items()}
    cb = inp["codebooks"].astype(np.float32)
    shared = {}
    import ml_dtypes
    for i in range(4):
        w = inp[f"enc_w{i}"].astype(np.float32)
        if i < 2:  # split layers: bf16 hi/lo weight pairs
            wh = w.astype(ml_dtypes.bfloat16)
            wl = (w - wh.astype(np.float32)).astype(ml_dtypes.bfloat16)
            shared[f"ewh{i}"] = _stripes(wh)
            shared[f"ewl{i}"] = _stripes(wl)
        else:
            shared[f"ew{i}"] = _stripes(w)
        shared[f"eb{i}"] = _bias_cols(inp[f"enc_b{i}"].astype(np.float32))
        shared[f"dw{i}"] = _stripes(
            inp[f"dec_w{i}"].astype(np.float32).astype(ml_dtypes.bfloat16))
        shared[f"db{i}"] = _bias_cols(inp[f"dec_b{i}"].astype(np.float32))
    shared["cbT2"] = np.ascontiguousarray(
        (2.0 * cb).transpose(0, 2, 1)).astype(np.float32)
    shared["cbN"] = cb
    c2 = (cb * cb).sum(-1)  # [4, 256] fp32
    shared["nc2b"] = np.ascontiguousarray(
        np.broadcast_to((-c2)[:, None, :], (4, P, 256))).astype(np.float32)
    a_raw = inp["a_raw"].astype(np.float32)
    b_raw = inp["b_raw"].astype(np.float32)
    a = np.log1p(np.exp(a_raw)).astype(np.float32) + np.float32(EPS)
    b = np.log1p(np.exp(b_raw)).astype(np.float32) + np.float32(EPS)
    shared["abp"] = np.stack(
        [a, b, (np.float32(1.0) / a), (np.float32(1.0) / b)],
        axis=1).astype(np.float32)
    shared["ident"] = np.eye(P, dtype=np.float32)
    return inp["x"].astype(np.float32), shared


def combine_outputs(results, bc, btot):
    out = np.concatenate([r["out"] for r in results], axis=0)
    indices = np.concatenate([r["indices"] for r in results], axis=0)
    sse = np.zeros((P, 8), np.float64)
    for r in results:
        sse += r["losses"].astype(np.float64)
    nel = float(btot * 128)
    rq = np.mean([(1.0 + BETA) * sse[:, l].sum() / nel for l in range(4)])
    nvq = sse[:, 4].sum() / nel
    loss = np.float32(rq + nvq)
    return out, loss, indices.astype(np.int32)


def run(inputs, trace=False):
    from concourse.bass_utils import run_bass_kernel_spmd
    x, shared = prep_inputs(inputs)
    nc = build_program(BC)
    in_maps = []
    for c in range(NCORES):
        m = dict(shared)
        m["x"] = np.ascontiguousarray(x[c * BC:(c + 1) * BC])
        in_maps.append(m)
    res = run_bass_kernel_spmd(nc, in_maps, list(range(NCORES)), trace=trace)
    out, loss, indices = combine_outputs(res.results, BC, B_FULL)
    return (out, loss, indices), res, nc


def kernel(**inputs):
    (out, loss, indices), _, _ = run(inputs, trace=False)
    return out, loss, indices
